# revision 8
# baseline (speedup 1.0000x reference)
"""Multi-head attention + residual + LayerNorm, Trainium2 Bass kernel.

Problem (hardcoded): B=8, S=2048, D=512, H=8, DK=64, fp32 I/O.
  q = Q@Wq.T+bq; k = K@Wk.T+bk; v = V@Wv.T+bv        (per batch, split 8 heads)
  attn = softmax(q k^T / sqrt(DK)); ctx = attn @ v
  out = LayerNorm(ctx@Wo.T + bo + Q) * gamma + beta

Sharding: pure data-parallel over batch: core b handles batch element b
(B == n_cores == 8), no collectives.

Per-core dataflow (t-major attention, fp8 DoubleRow matmuls, fp32 LN):
  - Q/K/V pre-transposed AND fp8(e4m3)-cast on host to kc-paired k-major
    layout [2, 128, 2, S]; projections run as fp8 DoubleRow matmuls
    (K=256 contraction per instruction = 2x bf16 column throughput).
  - qT,kT projections output bf16 [d_out, s]; v outputs to v8p fp8 tiles
    [128t, 2(t-parity), 8*128] where each head block is [dv(64)|ones(64)]
    for even heads and [ones|dv] for odd heads.  The ones columns fold
    the softmax denominator into the ctx matmul (cost is per-output-
    column, so 64 redundant den rows are free).
  - Attention per head pair p, s-block (512):
      scoresT[t,s] bf16 matmuls into a shared [128,1024] PSUM pair
      (head A cols 0:512, head B 512:1024),
      exp: head A true exp on ACT -> fp8, head B Schraudolph int8
      bit-trick on DVE -> fp8,
      ctx+den: ONE fp8 DoubleRow matmul per head per t-chunk-PAIR
      (contraction 256 t's) accumulating [dv|den] in PSUM.
  - Boundary per (pair, s-block): reciprocal of den row on DVE,
    PE outer-product broadcasts 1/den into the other head's dead den
    rows (partition-aligned), Pool-engine multiply normalizes into
    bf16 ctxT.  No DRAM roundtrips.
  - Output projection, +bias, +residual (fp32r identity matmul on
    prefetched Q tiles), LayerNorm, gamma/beta, DMA out.

Toolchain workarounds: this walrus build caps sem-waits per instruction
at 1 (excess waits hoisted onto same-engine NOPs).
"""

import numpy as np
import ml_dtypes

import bass_rust
import concourse.bass as bass
import concourse.mybir as mybir
import concourse.tile as tile
from concourse.bass_utils import run_bass_kernel_spmd
from concourse.vector_clock import ScopedClock

F32 = mybir.dt.float32
F32R = mybir.dt.float32r
BF16 = mybir.dt.bfloat16
FP8 = mybir.dt.float8e4
I8 = mybir.dt.int8
AF = mybir.ActivationFunctionType
OP = mybir.AluOpType
PM = mybir.MatmulPerfMode

N_CORES = 8
S, D, H, DK = 2048, 512, 8, 64
P = 128
KC = D // P        # 4 contraction chunks
TC = S // P        # 16 t-chunks
TP = TC // 2       # 8 t-chunk pairs
ST = S // P        # 16 s-tiles (output)
SBW = 512          # attention s-block width
NSB = S // SBW     # 4
EPS = 1e-5
SCALE = 1.0 / np.sqrt(DK)

# Schraudolph exp in fp8e4m3-bit space: bits = round(x*SCALE*8/ln2 + (56-c))
SCH8_S = float(SCALE * 8.0 / np.log(2.0))
SCH8_B = float(7 * 8 - 0.35)

_MAX_CTRL_WAITS = 1


def _patch_tile_tail():
    """walrus in this toolchain rejects >1 sem wait on CTRL instructions
    (Drain/NoOp). Move the Tile tail-drain's waits onto a chain of NOPs,
    one wait each."""
    if getattr(tile.TileContext, "_tail_patched", False):
        return

    def _patched(self, tick_clock, wait_clock):
        nc = self.nc
        scratch = nc.sync.nop(nofuse=True, hint="tail_wait")
        wait_clock.add_sem_waits(
            scratch.ins, ScopedClock({None: tick_clock.global_clock})
        )
        si = scratch.ins.sync_info
        waits = list(si.on_wait) if si is not None else []
        if len(waits) > _MAX_CTRL_WAITS:
            scratch.ins.sync_info = bass_rust.SyncInfo(
                on_wait=waits[:_MAX_CTRL_WAITS], on_update=list(si.on_update)
            )
            for i in range(_MAX_CTRL_WAITS, len(waits), _MAX_CTRL_WAITS):
                extra = nc.sync.nop(nofuse=True, hint=f"tail_wait_{i}")
                extra.ins.sync_info = bass_rust.SyncInfo(
                    on_wait=waits[i : i + _MAX_CTRL_WAITS], on_update=[]
                )
        nc.sync.drain()
        nc.all_engine_barrier()
        popped = nc._tile_sem_poison_stack.pop()
        assert popped is self._sem_poison
        nc.clear_and_free_semaphores(list(self.sems.allocated().values()))
        nc.all_engine_barrier()

    tile.TileContext._drain_and_barrier = _patched
    tile.TileContext._tail_patched = True


def _split_excess_waits(nc, max_waits=_MAX_CTRL_WAITS):
    """walrus (this build) caps sem waits per instruction very low. Hoist
    excess waits onto same-engine NOPs inserted just before the instruction
    (same queue, in order — semantically identical)."""
    def make_nop(engine, waits):
        bi = nc.engines[engine].nop(nofuse=True, hint="waitsplit")
        nop_inst = bi.ins
        cur = nc.cur_bb.bb
        lst = list(cur.instructions)
        assert lst and lst[-1].name == nop_inst.name
        lst.pop()
        cur.instructions = lst
        nop_inst.sync_info = bass_rust.SyncInfo(on_wait=waits, on_update=[])
        return nop_inst

    ctr = 0
    for f in nc.m.functions:
        for bb in f.blocks:
            old = list(bb.instructions)
            new = []
            changed = False
            for inst in old:
                si = inst.sync_info
                waits = list(si.on_wait) if si is not None else []
                if len(waits) > max_waits:
                    changed = True
                    excess, keep = waits[:-max_waits], waits[-max_waits:]
                    for i in range(0, len(excess), max_waits):
                        ctr += 1
                        new.append(make_nop(inst.engine, excess[i : i + max_waits]))
                    inst.sync_info = bass_rust.SyncInfo(
                        on_wait=keep, on_update=list(si.on_update)
                    )
                new.append(inst)
            if changed:
                bb.instructions = new
    return ctr


def build_program():
    _patch_tile_tail()
    nc = bass.Bass("TRN2", target_bir_lowering=False, debug=False, num_devices=1)

    qf = nc.dram_tensor("qf", (S, D), F32R, kind="ExternalInput").ap()
    qb8 = nc.dram_tensor("qb8", (2, P, 2, S), FP8, kind="ExternalInput").ap()
    kb8 = nc.dram_tensor("kb8", (2, P, 2, S), FP8, kind="ExternalInput").ap()
    vb8 = nc.dram_tensor("vb8", (2, P, 2, S), FP8, kind="ExternalInput").ap()
    wq8 = nc.dram_tensor("wq8", (2, P, 2, D), FP8, kind="ExternalInput").ap()
    wk8 = nc.dram_tensor("wk8", (2, P, 2, D), FP8, kind="ExternalInput").ap()
    wv8 = nc.dram_tensor("wv8", (2, P, 2, D), FP8, kind="ExternalInput").ap()
    wo = nc.dram_tensor("wo", (D, D), BF16, kind="ExternalInput").ap()
    bq = nc.dram_tensor("bq", (D,), F32, kind="ExternalInput").ap()
    bk = nc.dram_tensor("bk", (D,), F32, kind="ExternalInput").ap()
    bv = nc.dram_tensor("bv", (1, D), BF16, kind="ExternalInput").ap()
    bo = nc.dram_tensor("bo", (1, D), BF16, kind="ExternalInput").ap()
    gamma = nc.dram_tensor("gamma", (D,), F32, kind="ExternalInput").ap()
    beta = nc.dram_tensor("beta", (D,), F32, kind="ExternalInput").ap()
    ident = nc.dram_tensor("ident", (P, P), F32R, kind="ExternalInput").ap()
    out = nc.dram_tensor("out", (S, D), F32, kind="ExternalOutput").ap()
    # DRAM scratch for per-(pair, s-block) softmax-recip rows (partition-
    # broadcast DMA needs a DRAM source).
    dscr = nc.dram_tensor("dscr", (KC * NSB, 2, SBW), F32, kind="Internal").ap()

    with tile.TileContext(nc) as tc:
        with tc.tile_pool(name="persist", bufs=1) as pp:
            # ---- constants / weights ----
            wq_sb, wk_sb, wv_sb = [], [], []
            for j in range(2):
                t = pp.tile([P, 2, D], FP8, name=f"wq{j}")
                nc.sync.dma_start(out=t, in_=wq8[j])
                wq_sb.append(t)
                t = pp.tile([P, 2, D], FP8, name=f"wk{j}")
                nc.scalar.dma_start(out=t, in_=wk8[j])
                wk_sb.append(t)
                t = pp.tile([P, 2, D], FP8, name=f"wv{j}")
                nc.gpsimd.dma_start(out=t, in_=wv8[j])
                wv_sb.append(t)
            wo_sb = []
            for c in range(KC):
                t = pp.tile([P, D], BF16, name=f"wo{c}")
                nc.gpsimd.dma_start(out=t, in_=wo[c * P : (c + 1) * P, :])
                wo_sb.append(t)
            bq_sb, bk_sb = [], []
            for c in range(KC):
                t = pp.tile([P, 1], F32, name=f"bq{c}")
                nc.gpsimd.dma_start(out=t, in_=bq[c * P : (c + 1) * P].unsqueeze(1))
                bq_sb.append(t)
                t = pp.tile([P, 1], F32, name=f"bk{c}")
                nc.gpsimd.dma_start(out=t, in_=bk[c * P : (c + 1) * P].unsqueeze(1))
                bk_sb.append(t)
            bv_sb = pp.tile([1, D], BF16, name="bv")
            nc.gpsimd.dma_start(out=bv_sb, in_=bv)
            bo_sb = pp.tile([1, D], BF16, name="bo")
            nc.gpsimd.dma_start(out=bo_sb, in_=bo)
            gamma_sb = pp.tile([P, D], F32, name="gamma")
            nc.gpsimd.dma_start(out=gamma_sb, in_=gamma.unsqueeze(0).broadcast_to([P, D]))
            beta_sb = pp.tile([P, D], F32, name="beta")
            nc.gpsimd.dma_start(out=beta_sb, in_=beta.unsqueeze(0).broadcast_to([P, D]))
            eps_sb = pp.tile([P, 1], F32, name="eps")
            nc.vector.memset(eps_sb, EPS)
            ones_r = pp.tile([1, P], BF16, name="ones_r")
            nc.vector.memset(ones_r, 1.0)
            ident_sb = pp.tile([P, P], F32R, name="ident")
            nc.sync.dma_start(out=ident_sb, in_=ident)

            # ---- phase A: fp8 kc-paired k-major input loads ----
            QT8, KT8, VT8 = [], [], []
            for lst, srcap, nm, eng in (
                (QT8, qb8, "QT", nc.sync),
                (KT8, kb8, "KT", nc.scalar),
                (VT8, vb8, "VT", nc.gpsimd),
            ):
                for j in range(2):
                    t = pp.tile([P, 2, S], FP8, name=f"{nm}{j}")
                    eng.dma_start(out=t, in_=srcap[j])
                    lst.append(t)

            # residual prefetch (consumed in phase D)
            qres = []
            for st in range(ST):
                t = pp.tile([P, D], F32R, name=f"qres{st}")
                nc.sync.dma_start(out=t, in_=qf[st * P : (st + 1) * P, :])
                qres.append(t)

            # ---- phase B: projections (fp8 DoubleRow) ----
            qTp = [pp.tile([P, S], BF16, name=f"qTp{c}") for c in range(KC)]
            kTp = [pp.tile([P, S], BF16, name=f"kTp{c}") for c in range(KC)]
            # v8p[i]: [128t, 2(t-parity), 8 head blocks x 128]
            #   even head block: [dv(64) | ones(64)]; odd: [ones | dv]
            v8p = [pp.tile([P, 2, H * P], FP8, name=f"v8p{i}") for i in range(TP)]
            for i in range(TP):
                nc.vector.memset(v8p[i], 1.0)

            with tc.tile_pool(name="psum_b", bufs=4, space="PSUM") as ppool:
                for c in range(KC):
                    csl = bass.ts(c, P)
                    for sbh in range(NSB):
                        ssl = bass.ts(sbh, SBW)
                        pq = ppool.tile([P, SBW], F32, name="proj")
                        for j in range(2):
                            nc.tensor.matmul(
                                pq,
                                lhsT=wq_sb[j][:, :, csl],
                                rhs=QT8[j][:, :, ssl],
                                start=(j == 0),
                                stop=(j == 1),
                                perf_mode=PM.DoubleRow,
                            )
                        nc.scalar.activation(
                            out=qTp[c][:, ssl], in_=pq, func=AF.Identity, bias=bq_sb[c]
                        )
                        pk = ppool.tile([P, SBW], F32, name="proj")
                        for j in range(2):
                            nc.tensor.matmul(
                                pk,
                                lhsT=wk_sb[j][:, :, csl],
                                rhs=KT8[j][:, :, ssl],
                                start=(j == 0),
                                stop=(j == 1),
                                perf_mode=PM.DoubleRow,
                            )
                        nc.vector.tensor_scalar(
                            out=kTp[c][:, ssl],
                            in0=pk,
                            scalar1=bk_sb[c],
                            scalar2=None,
                            op0=OP.add,
                        )
                for t in range(TC):
                    pv = ppool.tile([P, 4, 2, 64], F32, name="pv")
                    for j in range(2):
                        nc.tensor.matmul(
                            pv,
                            lhsT=VT8[j][:, :, bass.ts(t, P)],
                            rhs=wv_sb[j],
                            start=(j == 0),
                            stop=False,
                            perf_mode=PM.DoubleRow,
                        )
                    nc.tensor.matmul(pv, lhsT=ones_r, rhs=bv_sb, start=False, stop=True)
                    v8r = v8p[t // 2][:, (t % 2) : (t % 2) + 1, :].rearrange(
                        "p x (a b c) -> p (x a) b c", a=4, b=2, c=P
                    )
                    nc.vector.tensor_copy(out=v8r[:, :, 0:1, 0:64], in_=pv[:, :, 0:1, :])
                    nc.vector.tensor_copy(out=v8r[:, :, 1:2, 64:128], in_=pv[:, :, 1:2, :])

            # ---- phase C: attention (per head pair p: heads 2p, 2p+1) ----
            ctxT = [pp.tile([P, S], BF16, name=f"ctxT{c}") for c in range(KC)]

            with (
                tc.tile_pool(name="psum_sc", bufs=2, space="PSUM") as psc,
                tc.tile_pool(name="psum_cd", bufs=2, space="PSUM") as pcd,
                tc.tile_pool(name="attn", bufs=3) as apool,
                tc.tile_pool(name="denr", bufs=2) as dpool,
            ):
                for p in range(KC):
                    dvA = bass.ds(2 * p * P, P)
                    dvB = bass.ds((2 * p + 1) * P, P)
                    for sb in range(NSB):
                        ssl = bass.ts(sb, SBW)
                        ctx2 = pcd.tile([P, 2 * SBW], F32, name="ctx2")
                        for i in range(TP):
                            e8A = apool.tile([P, 2, SBW], FP8, name="e8A")
                            e8B = apool.tile([P, 2, SBW], FP8, name="e8B")
                            for j in range(2):
                                tsl = bass.ts(2 * i + j, P)
                                sc2 = psc.tile([P, 2 * SBW], F32, name="sc2")
                                nc.tensor.matmul(
                                    sc2[:, 0:SBW],
                                    lhsT=kTp[p][0:DK, tsl],
                                    rhs=qTp[p][0:DK, ssl],
                                    start=True,
                                    stop=True,
                                    tile_position=(0, 0),
                                )
                                nc.tensor.matmul(
                                    sc2[:, SBW : 2 * SBW],
                                    lhsT=kTp[p][DK:P, tsl],
                                    rhs=qTp[p][DK:P, ssl],
                                    start=True,
                                    stop=True,
                                    tile_position=(64, 0),
                                )
                                nc.scalar.activation(
                                    out=e8A[:, j, :],
                                    in_=sc2[:, 0:SBW],
                                    func=AF.Exp,
                                    scale=SCALE,
                                )
                                nc.vector.tensor_scalar(
                                    out=e8B[:, j, :].bitcast(I8),
                                    in0=sc2[:, SBW : 2 * SBW],
                                    scalar1=SCH8_S,
                                    scalar2=SCH8_B,
                                    op0=OP.mult,
                                    op1=OP.add,
                                )
                            nc.tensor.matmul(
                                ctx2[:, 0:SBW],
                                lhsT=v8p[i][:, :, dvA],
                                rhs=e8A,
                                start=(i == 0),
                                stop=(i == TP - 1),
                                perf_mode=PM.DoubleRow,
                            )
                            nc.tensor.matmul(
                                ctx2[:, SBW : 2 * SBW],
                                lhsT=v8p[i][:, :, dvB],
                                rhs=e8B,
                                start=(i == 0),
                                stop=(i == TP - 1),
                                perf_mode=PM.DoubleRow,
                            )
                        # boundary: den -> 1/den on DVE, DRAM-roundtrip
                        # partition-broadcast (gpsimd queue), DVE normalize
                        # (one PSUM operand + SBUF broadcast) into bf16 ctxT.
                        u = p * NSB + sb
                        rec = dpool.tile([P, SBW], F32, name="rec")
                        nc.vector.reciprocal(
                            out=rec[64:65, :], in_=ctx2[64:65, 0:SBW]
                        )
                        nc.vector.reciprocal(
                            out=rec[0:1, :], in_=ctx2[0:1, SBW : 2 * SBW]
                        )
                        nc.gpsimd.dma_start(out=dscr[u, 0, :], in_=rec[64:65, :])
                        nc.gpsimd.dma_start(out=dscr[u, 1, :], in_=rec[0:1, :])
                        rbs = dpool.tile([P, SBW], F32, name="rbs")
                        nc.gpsimd.dma_start(
                            out=rbs[0:64, :],
                            in_=dscr[u, 0, :].unsqueeze(0).broadcast_to([64, SBW]),
                        )
                        nc.gpsimd.dma_start(
                            out=rbs[64:P, :],
                            in_=dscr[u, 1, :].unsqueeze(0).broadcast_to([64, SBW]),
                        )
                        nc.vector.tensor_tensor(
                            out=ctxT[p][0:64, ssl],
                            in0=ctx2[0:64, 0:SBW],
                            in1=rbs[0:64, :],
                            op=OP.mult,
                        )
                        nc.vector.tensor_tensor(
                            out=ctxT[p][64:P, ssl],
                            in0=ctx2[64:P, SBW : 2 * SBW],
                            in1=rbs[64:P, :],
                            op=OP.mult,
                        )

            # ---- phase D: output projection, residual, LN ----
            with (
                tc.tile_pool(name="psum_o", bufs=3, space="PSUM") as pout,
                tc.tile_pool(name="work", bufs=3) as wpool,
            ):
                for st in range(ST):
                    stsl = bass.ts(st, P)
                    po = pout.tile([P, D], F32, name="pout")
                    for c in range(KC):
                        nc.tensor.matmul(
                            po,
                            lhsT=ctxT[c][:, stsl],
                            rhs=wo_sb[c],
                            start=(c == 0),
                            stop=False,
                        )
                    nc.tensor.matmul(
                        po, lhsT=ones_r, rhs=bo_sb, start=False, stop=False
                    )
                    nc.tensor.matmul(
                        po,
                        lhsT=ident_sb[:],
                        rhs=qres[st][:],
                        start=False,
                        stop=True,
                    )
                    stats = wpool.tile([P, 6], F32, name="stats")
                    nc.vector.bn_stats(out=stats, in_=po)
                    mv = wpool.tile([P, 2], F32, name="mv")
                    nc.vector.bn_aggr(out=mv, in_=stats)
                    sq = wpool.tile([P, 1], F32, name="sq")
                    nc.scalar.activation(
                        out=sq, in_=mv[:, 1:2], func=AF.Sqrt, bias=eps_sb
                    )
                    rstd = wpool.tile([P, 1], F32, name="rstd")
                    nc.vector.reciprocal(out=rstd, in_=sq)
                    negmu = wpool.tile([P, 1], F32, name="negmu")
                    nc.vector.tensor_scalar(
                        out=negmu,
                        in0=mv[:, 0:1],
                        scalar1=rstd,
                        scalar2=-1.0,
                        op0=OP.mult,
                        op1=OP.mult,
                    )
                    x = wpool.tile([P, D], F32, name="x")
                    nc.scalar.activation(
                        out=x, in_=po, func=AF.Identity, bias=negmu, scale=rstd
                    )
                    nc.vector.tensor_mul(out=x, in0=x, in1=gamma_sb)
                    nc.gpsimd.tensor_tensor(
                        out=x, in0=x, in1=beta_sb, op=OP.add
                    )
                    nc.sync.dma_start(out=out[st * P : (st + 1) * P, :], in_=x)

    _split_excess_waits(nc)
    return nc


_NC_CACHE = None


def _get_program():
    global _NC_CACHE
    if _NC_CACHE is None:
        _NC_CACHE = build_program()
    return _NC_CACHE


def _pair_kc(xT):
    """[D, S]-like array -> kc-paired [2, 128, 2, S] layout."""
    d, s = xT.shape
    return np.ascontiguousarray(
        xT.reshape(2, 2, P, s).transpose(0, 2, 1, 3)
    )


def make_in_maps(Q, K, V, Wq, bq, Wk, bk, Wv, bv, Wo, bo, gamma, beta):
    bf = ml_dtypes.bfloat16
    f8 = ml_dtypes.float8_e4m3fn
    Q = np.asarray(Q, np.float32)
    K = np.asarray(K, np.float32)
    V = np.asarray(V, np.float32)
    wq8 = _pair_kc(np.asarray(Wq, np.float32).T).astype(f8)
    wk8 = _pair_kc(np.asarray(Wk, np.float32).T).astype(f8)
    wv8 = _pair_kc(np.asarray(Wv, np.float32).T).astype(f8)
    woT = np.ascontiguousarray(np.asarray(Wo, np.float32).T).astype(bf)
    bv_r = np.asarray(bv, np.float32).reshape(1, D).astype(bf)
    bo_r = np.asarray(bo, np.float32).reshape(1, D).astype(bf)
    ident = np.eye(P, dtype=np.float32)
    in_maps = []
    for b in range(N_CORES):
        in_maps.append(
            {
                "qf": np.ascontiguousarray(Q[b]),
                "qb8": _pair_kc(Q[b].T).astype(f8),
                "kb8": _pair_kc(K[b].T).astype(f8),
                "vb8": _pair_kc(V[b].T).astype(f8),
                "wq8": wq8,
                "wk8": wk8,
                "wv8": wv8,
                "wo": woT,
                "bq": np.asarray(bq, np.float32),
                "bk": np.asarray(bk, np.float32),
                "bv": bv_r,
                "bo": bo_r,
                "ident": ident,
                "gamma": np.asarray(gamma, np.float32),
                "beta": np.asarray(beta, np.float32),
            }
        )
    return in_maps


def run(in_maps, trace=False, **kw):
    nc = _get_program()
    return run_bass_kernel_spmd(
        nc, in_maps, core_ids=list(range(N_CORES)), trace=trace, **kw
    )


def kernel(**inputs):
    in_maps = make_in_maps(**inputs)
    res = run(in_maps)
    out = np.stack([res.results[b]["out"] for b in range(N_CORES)], axis=0)
    return out.astype(np.float32)


# revision 15
# speedup vs baseline: 1.0710x; 1.0710x over previous
"""Multi-head attention + residual + LayerNorm, Trainium2 Bass kernel.

Problem (hardcoded): B=8, S=2048, D=512, H=8, DK=64, fp32 I/O.
  q = Q@Wq.T+bq; k = K@Wk.T+bk; v = V@Wv.T+bv        (per batch, split 8 heads)
  attn = softmax(q k^T / sqrt(DK)); ctx = attn @ v
  out = LayerNorm(ctx@Wo.T + bo + Q) * gamma + beta

Sharding: pure data-parallel over batch: core b handles batch element b
(B == n_cores == 8), no collectives.

Per-core dataflow (t-major attention, fp8 DoubleRow matmuls, fp32 LN):
  - Q/K/V pre-transposed AND fp8(e4m3)-cast on host to kc-paired k-major
    layout [2, 128, 2, S]; projections run as fp8 DoubleRow matmuls
    (K=256 contraction per instruction = 2x bf16 column throughput).
  - qT,kT projections output bf16 [d_out, s]; v outputs to v8p fp8 tiles
    [128t, 2(t-parity), 8*128] where each head block is [dv(64)|ones(64)]
    for even heads and [ones|dv] for odd heads.  The ones columns fold
    the softmax denominator into the ctx matmul (cost is per-output-
    column, so 64 redundant den rows are free).
  - Attention per head pair p, s-block (512):
      scoresT[t,s] bf16 matmuls into a shared [128,1024] PSUM pair
      (head A cols 0:512, head B 512:1024),
      exp: head A true exp on ACT -> fp8, head B Schraudolph int8
      bit-trick on DVE -> fp8,
      ctx+den: ONE fp8 DoubleRow matmul per head per t-chunk-PAIR
      (contraction 256 t's) accumulating [dv|den] in PSUM.
  - Boundary per (pair, s-block): reciprocal of den row on DVE,
    PE outer-product broadcasts 1/den into the other head's dead den
    rows (partition-aligned), Pool-engine multiply normalizes into
    bf16 ctxT.  No DRAM roundtrips.
  - Output projection, +bias, +residual (fp32r identity matmul on
    prefetched Q tiles), LayerNorm, gamma/beta, DMA out.

Toolchain workarounds: this walrus build caps sem-waits per instruction
at 1 (excess waits hoisted onto same-engine NOPs).
"""

import numpy as np
import ml_dtypes

import bass_rust
import concourse.bass as bass
import concourse.mybir as mybir
import concourse.tile as tile
from concourse.bass_utils import run_bass_kernel_spmd
from concourse.vector_clock import ScopedClock

F32 = mybir.dt.float32
F32R = mybir.dt.float32r
BF16 = mybir.dt.bfloat16
FP8 = mybir.dt.float8e4
I8 = mybir.dt.int8
AF = mybir.ActivationFunctionType
OP = mybir.AluOpType
PM = mybir.MatmulPerfMode

N_CORES = 8
S, D, H, DK = 2048, 512, 8, 64
P = 128
KC = D // P        # 4 contraction chunks
TC = S // P        # 16 t-chunks
TP = TC // 2       # 8 t-chunk pairs
ST = S // P        # 16 s-tiles (output)
SBW = 512          # attention s-block width
NSB = S // SBW     # 4
EPS = 1e-5
SCALE = 1.0 / np.sqrt(DK)

# Schraudolph exp in fp8e4m3-bit space: bits = round(x*SCALE*8/ln2 + (56-c))
SCH8_S = float(SCALE * 8.0 / np.log(2.0))
SCH8_B = float(7 * 8 - 0.35)

_MAX_CTRL_WAITS = 1


def _patch_tile_tail():
    """walrus in this toolchain rejects >1 sem wait on CTRL instructions
    (Drain/NoOp). Move the Tile tail-drain's waits onto a chain of NOPs,
    one wait each."""
    if getattr(tile.TileContext, "_tail_patched", False):
        return

    def _patched(self, tick_clock, wait_clock):
        nc = self.nc
        scratch = nc.sync.nop(nofuse=True, hint="tail_wait")
        wait_clock.add_sem_waits(
            scratch.ins, ScopedClock({None: tick_clock.global_clock})
        )
        si = scratch.ins.sync_info
        waits = list(si.on_wait) if si is not None else []
        if len(waits) > _MAX_CTRL_WAITS:
            scratch.ins.sync_info = bass_rust.SyncInfo(
                on_wait=waits[:_MAX_CTRL_WAITS], on_update=list(si.on_update)
            )
            for i in range(_MAX_CTRL_WAITS, len(waits), _MAX_CTRL_WAITS):
                extra = nc.sync.nop(nofuse=True, hint=f"tail_wait_{i}")
                extra.ins.sync_info = bass_rust.SyncInfo(
                    on_wait=waits[i : i + _MAX_CTRL_WAITS], on_update=[]
                )
        nc.sync.drain()
        nc.all_engine_barrier()
        popped = nc._tile_sem_poison_stack.pop()
        assert popped is self._sem_poison
        nc.clear_and_free_semaphores(list(self.sems.allocated().values()))
        nc.all_engine_barrier()

    tile.TileContext._drain_and_barrier = _patched
    tile.TileContext._tail_patched = True


def _split_excess_waits(nc, max_waits=_MAX_CTRL_WAITS):
    """walrus (this build) caps sem waits per instruction very low. Hoist
    excess waits onto same-engine NOPs inserted just before the instruction
    (same queue, in order — semantically identical)."""
    def make_nop(engine, waits):
        bi = nc.engines[engine].nop(nofuse=True, hint="waitsplit")
        nop_inst = bi.ins
        cur = nc.cur_bb.bb
        lst = list(cur.instructions)
        assert lst and lst[-1].name == nop_inst.name
        lst.pop()
        cur.instructions = lst
        nop_inst.sync_info = bass_rust.SyncInfo(on_wait=waits, on_update=[])
        return nop_inst

    ctr = 0
    for f in nc.m.functions:
        for bb in f.blocks:
            old = list(bb.instructions)
            new = []
            changed = False
            for inst in old:
                si = inst.sync_info
                waits = list(si.on_wait) if si is not None else []
                if len(waits) > max_waits:
                    changed = True
                    excess, keep = waits[:-max_waits], waits[-max_waits:]
                    for i in range(0, len(excess), max_waits):
                        ctr += 1
                        new.append(make_nop(inst.engine, excess[i : i + max_waits]))
                    inst.sync_info = bass_rust.SyncInfo(
                        on_wait=keep, on_update=list(si.on_update)
                    )
                new.append(inst)
            if changed:
                bb.instructions = new
    return ctr


_LDW_OPT = False


def _patch_ldw_opt():
    """Enable walrus's LDWEIGHTS pull-ahead (background weight buffer) —
    concourse pins it off, but it is a large win for a LDW-per-matmul
    instruction stream. Correctness is re-verified against the reference
    each run."""
    import concourse.bass_utils as bu

    if getattr(bu, "_ldw_patched", False):
        return
    orig = bu.run_command

    def patched(cmd, **kw):
        if _LDW_OPT and isinstance(cmd, list):
            cmd = [
                c.replace("--enable-ldw-opt=false", "--enable-ldw-opt=true")
                if isinstance(c, str)
                else c
                for c in cmd
            ]
        return orig(cmd, **kw)

    bu.run_command = patched
    bu._ldw_patched = True


def build_program():
    _patch_tile_tail()
    _patch_ldw_opt()
    nc = bass.Bass("TRN2", target_bir_lowering=False, debug=False, num_devices=1)

    qf = nc.dram_tensor("qf", (S, D), F32R, kind="ExternalInput").ap()
    qb8 = nc.dram_tensor("qb8", (2, P, 2, S), FP8, kind="ExternalInput").ap()
    kb8 = nc.dram_tensor("kb8", (2, P, 2, S), FP8, kind="ExternalInput").ap()
    vb8 = nc.dram_tensor("vb8", (2, P, 2, S), FP8, kind="ExternalInput").ap()
    wq8 = nc.dram_tensor("wq8", (2, P, 2, D), FP8, kind="ExternalInput").ap()
    wk8 = nc.dram_tensor("wk8", (2, P, 2, D), FP8, kind="ExternalInput").ap()
    wv8 = nc.dram_tensor("wv8", (2, P, 2, D), FP8, kind="ExternalInput").ap()
    wo = nc.dram_tensor("wo", (D, D), BF16, kind="ExternalInput").ap()
    bq = nc.dram_tensor("bq", (D,), F32, kind="ExternalInput").ap()
    bk = nc.dram_tensor("bk", (D,), F32, kind="ExternalInput").ap()
    bv = nc.dram_tensor("bv", (1, D), BF16, kind="ExternalInput").ap()
    bo = nc.dram_tensor("bo", (1, D), BF16, kind="ExternalInput").ap()
    gamma = nc.dram_tensor("gamma", (D,), F32, kind="ExternalInput").ap()
    beta = nc.dram_tensor("beta", (D,), F32, kind="ExternalInput").ap()
    ident = nc.dram_tensor("ident", (P, P), F32R, kind="ExternalInput").ap()
    out = nc.dram_tensor("out", (S, D), F32, kind="ExternalOutput").ap()
    # DRAM scratch for per-(pair, s-block) softmax denominators: written as
    # two [1,512] rows, read back as [128,8] (so the reciprocal runs on 128
    # DVE lanes instead of 1), recip written out, then partition-broadcast
    # back in (broadcast DMA needs a DRAM source).
    dscr = nc.dram_tensor("dscr", (KC * NSB, P, 8), F32, kind="Internal").ap()
    dscr2 = nc.dram_tensor("dscr2", (KC * NSB, P, 8), F32, kind="Internal").ap()

    with tile.TileContext(nc) as tc:
        with tc.tile_pool(name="persist", bufs=1) as pp:
            # ---- constants / weights ----
            wq_sb, wk_sb, wv_sb = [], [], []
            for j in range(2):
                t = pp.tile([P, 2, D], FP8, name=f"wq{j}")
                nc.sync.dma_start(out=t, in_=wq8[j])
                wq_sb.append(t)
                t = pp.tile([P, 2, D], FP8, name=f"wk{j}")
                nc.scalar.dma_start(out=t, in_=wk8[j])
                wk_sb.append(t)
                t = pp.tile([P, 2, D], FP8, name=f"wv{j}")
                nc.gpsimd.dma_start(out=t, in_=wv8[j])
                wv_sb.append(t)
            wo_sb = []
            for c in range(KC):
                t = pp.tile([P, D], BF16, name=f"wo{c}")
                nc.gpsimd.dma_start(out=t, in_=wo[c * P : (c + 1) * P, :])
                wo_sb.append(t)
            bq_sb, bk_sb = [], []
            for c in range(KC):
                t = pp.tile([P, 1], F32, name=f"bq{c}")
                nc.gpsimd.dma_start(out=t, in_=bq[c * P : (c + 1) * P].unsqueeze(1))
                bq_sb.append(t)
                t = pp.tile([P, 1], F32, name=f"bk{c}")
                nc.gpsimd.dma_start(out=t, in_=bk[c * P : (c + 1) * P].unsqueeze(1))
                bk_sb.append(t)
            bv_sb = pp.tile([1, D], BF16, name="bv")
            nc.gpsimd.dma_start(out=bv_sb, in_=bv)
            bo_sb = pp.tile([1, D], BF16, name="bo")
            nc.gpsimd.dma_start(out=bo_sb, in_=bo)
            gamma_sb = pp.tile([P, D], F32, name="gamma")
            nc.gpsimd.dma_start(out=gamma_sb, in_=gamma.unsqueeze(0).broadcast_to([P, D]))
            beta_sb = pp.tile([P, D], F32, name="beta")
            nc.gpsimd.dma_start(out=beta_sb, in_=beta.unsqueeze(0).broadcast_to([P, D]))
            eps_sb = pp.tile([P, 1], F32, name="eps")
            nc.vector.memset(eps_sb, EPS)
            ones_r = pp.tile([1, P], BF16, name="ones_r")
            nc.vector.memset(ones_r, 1.0)
            ident_sb = pp.tile([P, P], F32R, name="ident")
            nc.sync.dma_start(out=ident_sb, in_=ident)

            # ---- phase A: fp8 kc-paired k-major input loads ----
            QT8, KT8, VT8 = [], [], []
            for lst, srcap, nm, eng in (
                (QT8, qb8, "QT", nc.sync),
                (KT8, kb8, "KT", nc.scalar),
                (VT8, vb8, "VT", nc.gpsimd),
            ):
                for j in range(2):
                    t = pp.tile([P, 2, S], FP8, name=f"{nm}{j}")
                    eng.dma_start(out=t, in_=srcap[j])
                    lst.append(t)

            # residual prefetch (consumed in phase D)
            qres = []
            for st in range(ST):
                t = pp.tile([P, D], F32R, name=f"qres{st}")
                nc.sync.dma_start(out=t, in_=qf[st * P : (st + 1) * P, :])
                qres.append(t)

            # ---- phase B: projections (fp8 DoubleRow) ----
            qTp = [pp.tile([P, S], BF16, name=f"qTp{c}") for c in range(KC)]
            kTp = [pp.tile([P, S], BF16, name=f"kTp{c}") for c in range(KC)]
            # v8p[i]: [128t, 2(t-parity), 8 head blocks x 128]
            #   even head block: [dv(64) | ones(64)]; odd: [ones | dv]
            v8p = [pp.tile([P, 2, H * P], FP8, name=f"v8p{i}") for i in range(TP)]
            for i in range(TP):
                nc.vector.memset(v8p[i], 1.0)

            with tc.tile_pool(name="psum_b", bufs=4, space="PSUM") as ppool:
                for c in range(KC):
                    csl = bass.ts(c, P)
                    for sbh in range(NSB):
                        ssl = bass.ts(sbh, SBW)
                        pq = ppool.tile([P, SBW], F32, name="proj")
                        for j in range(2):
                            nc.tensor.matmul(
                                pq,
                                lhsT=wq_sb[j][:, :, csl],
                                rhs=QT8[j][:, :, ssl],
                                start=(j == 0),
                                stop=(j == 1),
                                perf_mode=PM.DoubleRow,
                            )
                        nc.scalar.activation(
                            out=qTp[c][:, ssl], in_=pq, func=AF.Identity, bias=bq_sb[c]
                        )
                        pk = ppool.tile([P, SBW], F32, name="proj")
                        for j in range(2):
                            nc.tensor.matmul(
                                pk,
                                lhsT=wk_sb[j][:, :, csl],
                                rhs=KT8[j][:, :, ssl],
                                start=(j == 0),
                                stop=(j == 1),
                                perf_mode=PM.DoubleRow,
                            )
                        nc.vector.tensor_scalar(
                            out=kTp[c][:, ssl],
                            in0=pk,
                            scalar1=bk_sb[c],
                            scalar2=None,
                            op0=OP.add,
                        )
                for t in range(TC):
                    pv = ppool.tile([P, 4, 2, 64], F32, name="pv")
                    for j in range(2):
                        nc.tensor.matmul(
                            pv,
                            lhsT=VT8[j][:, :, bass.ts(t, P)],
                            rhs=wv_sb[j],
                            start=(j == 0),
                            stop=False,
                            perf_mode=PM.DoubleRow,
                        )
                    nc.tensor.matmul(pv, lhsT=ones_r, rhs=bv_sb, start=False, stop=True)
                    v8r = v8p[t // 2][:, (t % 2) : (t % 2) + 1, :].rearrange(
                        "p x (a b c) -> p (x a) b c", a=4, b=2, c=P
                    )
                    nc.vector.tensor_copy(out=v8r[:, :, 0:1, 0:64], in_=pv[:, :, 0:1, :])
                    nc.vector.tensor_copy(out=v8r[:, :, 1:2, 64:128], in_=pv[:, :, 1:2, :])

            # ---- phase C: attention (per head pair p: heads 2p, 2p+1) ----
            ctxT = [pp.tile([P, S], BF16, name=f"ctxT{c}") for c in range(KC)]

            with (
                tc.tile_pool(name="psum_sc", bufs=2, space="PSUM") as psc,
                tc.tile_pool(name="psum_cd", bufs=2, space="PSUM") as pcd,
                tc.tile_pool(name="attn", bufs=3) as apool,
                tc.tile_pool(name="denr", bufs=2) as dpool,
            ):
                def _boundary(bp, bsb, bctx2):
                    # drain a finished (pair, s-block): den rows -> DRAM
                    # reshaped [128,8], reciprocal on 128 lanes, roundtrip
                    # back as partition-broadcast, normalize into ctxT.
                    # Called 2 t-pairs into the NEXT unit so this work queues
                    # BEHIND that unit's first exps and doesn't stall them.
                    u = bp * NSB + bsb
                    bsl = bass.ts(bsb, SBW)
                    # den rows to SBUF (DMA cannot read PSUM): A den at row
                    # 64 cols 0:512, B den at row 0 cols 512:1024.
                    stage = dpool.tile([P, 2 * SBW], F32, name="stage")
                    nc.scalar.activation(
                        out=stage[64:65, 0:SBW],
                        in_=bctx2[64:65, 0:SBW],
                        func=AF.Identity,
                    )
                    nc.scalar.activation(
                        out=stage[0:1, SBW : 2 * SBW],
                        in_=bctx2[0:1, SBW : 2 * SBW],
                        func=AF.Identity,
                    )
                    nc.gpsimd.dma_start(
                        out=dscr[u, 0:64, :].rearrange("p c -> (p c)").unsqueeze(0),
                        in_=stage[64:65, 0:SBW],
                    )
                    nc.gpsimd.dma_start(
                        out=dscr[u, 64:P, :].rearrange("p c -> (p c)").unsqueeze(0),
                        in_=stage[0:1, SBW : 2 * SBW],
                    )
                    recd = dpool.tile([P, 8], F32, name="recd")
                    nc.gpsimd.dma_start(out=recd, in_=dscr[u])
                    recr = dpool.tile([P, 8], F32, name="recr")
                    nc.vector.reciprocal(out=recr, in_=recd)
                    nc.gpsimd.dma_start(out=dscr2[u], in_=recr)
                    rbs = dpool.tile([P, SBW], F32, name="rbs")
                    nc.gpsimd.dma_start(
                        out=rbs[0:64, :],
                        in_=dscr2[u, 0:64, :]
                        .rearrange("p c -> (p c)")
                        .unsqueeze(0)
                        .broadcast_to([64, SBW]),
                    )
                    nc.gpsimd.dma_start(
                        out=rbs[64:P, :],
                        in_=dscr2[u, 64:P, :]
                        .rearrange("p c -> (p c)")
                        .unsqueeze(0)
                        .broadcast_to([64, SBW]),
                    )
                    nc.vector.tensor_tensor(
                        out=ctxT[bp][0:64, bsl],
                        in0=bctx2[0:64, 0:SBW],
                        in1=rbs[0:64, :],
                        op=OP.mult,
                    )
                    nc.vector.tensor_tensor(
                        out=ctxT[bp][64:P, bsl],
                        in0=bctx2[64:P, SBW : 2 * SBW],
                        in1=rbs[64:P, :],
                        op=OP.mult,
                    )

                _pending = None
                for p in range(KC):
                    dvA = bass.ds(2 * p * P, P)
                    dvB = bass.ds((2 * p + 1) * P, P)
                    for sb in range(NSB):
                        ssl = bass.ts(sb, SBW)
                        ctx2 = pcd.tile([P, 2 * SBW], F32, name="ctx2")
                        for i in range(TP):
                            if i == 2 and _pending is not None:
                                _boundary(*_pending)
                                _pending = None
                            e8A = apool.tile([P, 2, SBW], FP8, name="e8A")
                            e8B = apool.tile([P, 2, SBW], FP8, name="e8B")
                            for j in range(2):
                                tsl = bass.ts(2 * i + j, P)
                                sc2 = psc.tile([P, 2 * SBW], F32, name="sc2")
                                nc.tensor.matmul(
                                    sc2[:, 0:SBW],
                                    lhsT=kTp[p][0:DK, tsl],
                                    rhs=qTp[p][0:DK, ssl],
                                    start=True,
                                    stop=True,
                                    tile_position=(0, 0),
                                )
                                nc.tensor.matmul(
                                    sc2[:, SBW : 2 * SBW],
                                    lhsT=kTp[p][DK:P, tsl],
                                    rhs=qTp[p][DK:P, ssl],
                                    start=True,
                                    stop=True,
                                    tile_position=(64, 0),
                                )
                                nc.scalar.activation(
                                    out=e8A[:, j, :],
                                    in_=sc2[:, 0:SBW],
                                    func=AF.Exp,
                                    scale=SCALE,
                                )
                                nc.vector.tensor_scalar(
                                    out=e8B[:, j, :].bitcast(I8),
                                    in0=sc2[:, SBW : 2 * SBW],
                                    scalar1=SCH8_S,
                                    scalar2=SCH8_B,
                                    op0=OP.mult,
                                    op1=OP.add,
                                )
                            nc.tensor.matmul(
                                ctx2[:, 0:SBW],
                                lhsT=v8p[i][:, :, dvA],
                                rhs=e8A,
                                start=(i == 0),
                                stop=(i == TP - 1),
                                perf_mode=PM.DoubleRow,
                            )
                            nc.tensor.matmul(
                                ctx2[:, SBW : 2 * SBW],
                                lhsT=v8p[i][:, :, dvB],
                                rhs=e8B,
                                start=(i == 0),
                                stop=(i == TP - 1),
                                perf_mode=PM.DoubleRow,
                            )
                        _pending = (p, sb, ctx2)

                if _pending is not None:
                    _boundary(*_pending)
                    _pending = None

            # ---- phase D: output projection, residual, LN ----
            with (
                tc.tile_pool(name="psum_o", bufs=3, space="PSUM") as pout,
                tc.tile_pool(name="work", bufs=3) as wpool,
            ):
                for st in range(ST):
                    stsl = bass.ts(st, P)
                    po = pout.tile([P, D], F32, name="pout")
                    for c in range(KC):
                        nc.tensor.matmul(
                            po,
                            lhsT=ctxT[c][:, stsl],
                            rhs=wo_sb[c],
                            start=(c == 0),
                            stop=False,
                        )
                    nc.tensor.matmul(
                        po, lhsT=ones_r, rhs=bo_sb, start=False, stop=False
                    )
                    nc.tensor.matmul(
                        po,
                        lhsT=ident_sb[:],
                        rhs=qres[st][:],
                        start=False,
                        stop=True,
                    )
                    stats = wpool.tile([P, 6], F32, name="stats")
                    nc.vector.bn_stats(out=stats, in_=po)
                    mv = wpool.tile([P, 2], F32, name="mv")
                    nc.vector.bn_aggr(out=mv, in_=stats)
                    sq = wpool.tile([P, 1], F32, name="sq")
                    nc.scalar.activation(
                        out=sq, in_=mv[:, 1:2], func=AF.Sqrt, bias=eps_sb
                    )
                    rstd = wpool.tile([P, 1], F32, name="rstd")
                    nc.vector.reciprocal(out=rstd, in_=sq)
                    negmu = wpool.tile([P, 1], F32, name="negmu")
                    nc.vector.tensor_scalar(
                        out=negmu,
                        in0=mv[:, 0:1],
                        scalar1=rstd,
                        scalar2=-1.0,
                        op0=OP.mult,
                        op1=OP.mult,
                    )
                    x = wpool.tile([P, D], F32, name="x")
                    nc.scalar.activation(
                        out=x, in_=po, func=AF.Identity, bias=negmu, scale=rstd
                    )
                    nc.vector.tensor_mul(out=x, in0=x, in1=gamma_sb)
                    nc.gpsimd.tensor_tensor(
                        out=x, in0=x, in1=beta_sb, op=OP.add
                    )
                    nc.sync.dma_start(out=out[st * P : (st + 1) * P, :], in_=x)

    _split_excess_waits(nc)
    return nc


_NC_CACHE = None


def _get_program():
    global _NC_CACHE
    if _NC_CACHE is None:
        _NC_CACHE = build_program()
    return _NC_CACHE


def _pair_kc(xT):
    """[D, S]-like array -> kc-paired [2, 128, 2, S] layout."""
    d, s = xT.shape
    return np.ascontiguousarray(
        xT.reshape(2, 2, P, s).transpose(0, 2, 1, 3)
    )


def make_in_maps(Q, K, V, Wq, bq, Wk, bk, Wv, bv, Wo, bo, gamma, beta):
    bf = ml_dtypes.bfloat16
    f8 = ml_dtypes.float8_e4m3fn
    Q = np.asarray(Q, np.float32)
    K = np.asarray(K, np.float32)
    V = np.asarray(V, np.float32)
    wq8 = _pair_kc(np.asarray(Wq, np.float32).T).astype(f8)
    wk8 = _pair_kc(np.asarray(Wk, np.float32).T).astype(f8)
    wv8 = _pair_kc(np.asarray(Wv, np.float32).T).astype(f8)
    woT = np.ascontiguousarray(np.asarray(Wo, np.float32).T).astype(bf)
    bv_r = np.asarray(bv, np.float32).reshape(1, D).astype(bf)
    bo_r = np.asarray(bo, np.float32).reshape(1, D).astype(bf)
    ident = np.eye(P, dtype=np.float32)
    in_maps = []
    for b in range(N_CORES):
        in_maps.append(
            {
                "qf": np.ascontiguousarray(Q[b]),
                "qb8": _pair_kc(Q[b].T).astype(f8),
                "kb8": _pair_kc(K[b].T).astype(f8),
                "vb8": _pair_kc(V[b].T).astype(f8),
                "wq8": wq8,
                "wk8": wk8,
                "wv8": wv8,
                "wo": woT,
                "bq": np.asarray(bq, np.float32),
                "bk": np.asarray(bk, np.float32),
                "bv": bv_r,
                "bo": bo_r,
                "ident": ident,
                "gamma": np.asarray(gamma, np.float32),
                "beta": np.asarray(beta, np.float32),
            }
        )
    return in_maps


def run(in_maps, trace=False, **kw):
    nc = _get_program()
    return run_bass_kernel_spmd(
        nc, in_maps, core_ids=list(range(N_CORES)), trace=trace, **kw
    )


def kernel(**inputs):
    in_maps = make_in_maps(**inputs)
    res = run(in_maps)
    out = np.stack([res.results[b]["out"] for b in range(N_CORES)], axis=0)
    return out.astype(np.float32)


# revision 20
# speedup vs baseline: 1.3335x; 1.2451x over previous
"""Multi-head attention + residual + LayerNorm, Trainium2 Bass kernel.

Problem (hardcoded): B=8, S=2048, D=512, H=8, DK=64, fp32 I/O.
  q = Q@Wq.T+bq; k = K@Wk.T+bk; v = V@Wv.T+bv        (per batch, split 8 heads)
  attn = softmax(q k^T / sqrt(DK)); ctx = attn @ v
  out = LayerNorm(ctx@Wo.T + bo + Q) * gamma + beta

Sharding: pure data-parallel over batch: core b handles batch element b
(B == n_cores == 8), no collectives.

Per-core dataflow (t-major attention, fp8 DoubleRow matmuls, fp32 LN):
  - Q/K/V pre-transposed AND fp8(e4m3)-cast on host to kc-paired k-major
    layout [2, 128, 2, S]; projections run as fp8 DoubleRow matmuls
    (K=256 contraction per instruction = 2x bf16 column throughput).
  - qT,kT projections output bf16 [d_out, s]; v outputs to v8p fp8 tiles
    [128t, 2(t-parity), 8*128] where each head block is [dv(64)|ones(64)]
    for even heads and [ones|dv] for odd heads.  The ones columns fold
    the softmax denominator into the ctx matmul (cost is per-output-
    column, so 64 redundant den rows are free).
  - Attention per head pair p, s-block (512):
      scoresT[t,s] bf16 matmuls into a shared [128,1024] PSUM pair
      (head A cols 0:512, head B 512:1024),
      exp: head A true exp on ACT -> fp8, head B Schraudolph int8
      bit-trick on DVE -> fp8,
      ctx+den: ONE fp8 DoubleRow matmul per head per t-chunk-PAIR
      (contraction 256 t's) accumulating [dv|den] in PSUM.
  - Boundary per (pair, s-block): reciprocal of den row on DVE,
    PE outer-product broadcasts 1/den into the other head's dead den
    rows (partition-aligned), Pool-engine multiply normalizes into
    bf16 ctxT.  No DRAM roundtrips.
  - Output projection, +bias, +residual (fp32r identity matmul on
    prefetched Q tiles), LayerNorm, gamma/beta, DMA out.

Toolchain workarounds: this walrus build caps sem-waits per instruction
at 1 (excess waits hoisted onto same-engine NOPs).
"""

import numpy as np
import ml_dtypes

import bass_rust
import concourse.bass as bass
import concourse.mybir as mybir
import concourse.tile as tile
from concourse.bass_utils import run_bass_kernel_spmd
from concourse.vector_clock import ScopedClock

F32 = mybir.dt.float32
F32R = mybir.dt.float32r
BF16 = mybir.dt.bfloat16
FP8 = mybir.dt.float8e4
I8 = mybir.dt.int8
I16 = mybir.dt.int16
AF = mybir.ActivationFunctionType
OP = mybir.AluOpType
PM = mybir.MatmulPerfMode

N_CORES = 8
S, D, H, DK = 2048, 512, 8, 64
P = 128
KC = D // P        # 4 contraction chunks
TC = S // P        # 16 t-chunks
TP = TC // 2       # 8 t-chunk pairs
ST = S // P        # 16 s-tiles (output)
SBW = 512          # attention s-block width
NSB = S // SBW     # 4
EPS = 1e-5
SCALE = 1.0 / np.sqrt(DK)

# Schraudolph exp in bf16-bit space: bits = round(x*L*SCALE + (16256 - C))
SCH_L = 128.0 / np.log(2.0)
SCH_C = 5.60
SCH_S = float(SCALE * SCH_L)
SCH_B = float(16256.0 - SCH_C)

_MAX_CTRL_WAITS = 1


def _patch_tile_tail():
    """walrus in this toolchain rejects >1 sem wait on CTRL instructions
    (Drain/NoOp). Move the Tile tail-drain's waits onto a chain of NOPs,
    one wait each."""
    if getattr(tile.TileContext, "_tail_patched", False):
        return

    def _patched(self, tick_clock, wait_clock):
        nc = self.nc
        scratch = nc.sync.nop(nofuse=True, hint="tail_wait")
        wait_clock.add_sem_waits(
            scratch.ins, ScopedClock({None: tick_clock.global_clock})
        )
        si = scratch.ins.sync_info
        waits = list(si.on_wait) if si is not None else []
        if len(waits) > _MAX_CTRL_WAITS:
            scratch.ins.sync_info = bass_rust.SyncInfo(
                on_wait=waits[:_MAX_CTRL_WAITS], on_update=list(si.on_update)
            )
            for i in range(_MAX_CTRL_WAITS, len(waits), _MAX_CTRL_WAITS):
                extra = nc.sync.nop(nofuse=True, hint=f"tail_wait_{i}")
                extra.ins.sync_info = bass_rust.SyncInfo(
                    on_wait=waits[i : i + _MAX_CTRL_WAITS], on_update=[]
                )
        nc.sync.drain()
        nc.all_engine_barrier()
        popped = nc._tile_sem_poison_stack.pop()
        assert popped is self._sem_poison
        nc.clear_and_free_semaphores(list(self.sems.allocated().values()))
        nc.all_engine_barrier()

    tile.TileContext._drain_and_barrier = _patched
    tile.TileContext._tail_patched = True


def _split_excess_waits(nc, max_waits=_MAX_CTRL_WAITS):
    """walrus (this build) caps sem waits per instruction very low. Hoist
    excess waits onto same-engine NOPs inserted just before the instruction
    (same queue, in order — semantically identical)."""
    def make_nop(engine, waits):
        bi = nc.engines[engine].nop(nofuse=True, hint="waitsplit")
        nop_inst = bi.ins
        cur = nc.cur_bb.bb
        lst = list(cur.instructions)
        assert lst and lst[-1].name == nop_inst.name
        lst.pop()
        cur.instructions = lst
        nop_inst.sync_info = bass_rust.SyncInfo(on_wait=waits, on_update=[])
        return nop_inst

    ctr = 0
    for f in nc.m.functions:
        for bb in f.blocks:
            old = list(bb.instructions)
            new = []
            changed = False
            for inst in old:
                si = inst.sync_info
                waits = list(si.on_wait) if si is not None else []
                if len(waits) > max_waits:
                    changed = True
                    excess, keep = waits[:-max_waits], waits[-max_waits:]
                    for i in range(0, len(excess), max_waits):
                        ctr += 1
                        new.append(make_nop(inst.engine, excess[i : i + max_waits]))
                    inst.sync_info = bass_rust.SyncInfo(
                        on_wait=keep, on_update=list(si.on_update)
                    )
                new.append(inst)
            if changed:
                bb.instructions = new
    return ctr


_LDW_OPT = False


def _patch_ldw_opt():
    """Enable walrus's LDWEIGHTS pull-ahead (background weight buffer) —
    concourse pins it off, but it is a large win for a LDW-per-matmul
    instruction stream. Correctness is re-verified against the reference
    each run."""
    import concourse.bass_utils as bu

    if getattr(bu, "_ldw_patched", False):
        return
    orig = bu.run_command

    def patched(cmd, **kw):
        if _LDW_OPT and isinstance(cmd, list):
            cmd = [
                c.replace("--enable-ldw-opt=false", "--enable-ldw-opt=true")
                if isinstance(c, str)
                else c
                for c in cmd
            ]
        return orig(cmd, **kw)

    bu.run_command = patched
    bu._ldw_patched = True


def build_program():
    _patch_tile_tail()
    _patch_ldw_opt()
    nc = bass.Bass("TRN2", target_bir_lowering=False, debug=False, num_devices=1)

    qf = nc.dram_tensor("qf", (S, D), F32R, kind="ExternalInput").ap()
    qb8 = nc.dram_tensor("qb8", (2, P, 2, S), FP8, kind="ExternalInput").ap()
    kb8 = nc.dram_tensor("kb8", (2, P, 2, S), FP8, kind="ExternalInput").ap()
    vb8 = nc.dram_tensor("vb8", (2, P, 2, S), FP8, kind="ExternalInput").ap()
    wq8 = nc.dram_tensor("wq8", (2, P, 2, D), FP8, kind="ExternalInput").ap()
    wk8 = nc.dram_tensor("wk8", (2, P, 2, D), FP8, kind="ExternalInput").ap()
    wv8 = nc.dram_tensor("wv8", (2, P, 2, D), FP8, kind="ExternalInput").ap()
    wo = nc.dram_tensor("wo", (D, D), BF16, kind="ExternalInput").ap()
    bq = nc.dram_tensor("bq", (D,), F32, kind="ExternalInput").ap()
    bk = nc.dram_tensor("bk", (D,), F32, kind="ExternalInput").ap()
    bv = nc.dram_tensor("bv", (1, D), BF16, kind="ExternalInput").ap()
    bo = nc.dram_tensor("bo", (1, D), BF16, kind="ExternalInput").ap()
    gamma = nc.dram_tensor("gamma", (D,), F32, kind="ExternalInput").ap()
    beta = nc.dram_tensor("beta", (D,), F32, kind="ExternalInput").ap()
    ident = nc.dram_tensor("ident", (P, P), F32R, kind="ExternalInput").ap()
    out = nc.dram_tensor("out", (S, D), F32, kind="ExternalOutput").ap()
    # DRAM scratch for per-(pair, s-block) softmax denominators: written as
    # two [1,512] rows, read back as [128,8] (so the reciprocal runs on 128
    # DVE lanes instead of 1), recip written out, then partition-broadcast
    # back in (broadcast DMA needs a DRAM source).
    dscr = nc.dram_tensor("dscr", (KC * NSB, P, 8), F32, kind="Internal").ap()
    dscr2 = nc.dram_tensor("dscr2", (KC * NSB, P, 8), F32, kind="Internal").ap()

    with tile.TileContext(nc) as tc:
        with tc.tile_pool(name="persist", bufs=1) as pp:
            # ---- constants / weights ----
            wq_sb, wk_sb, wv_sb = [], [], []
            for j in range(2):
                t = pp.tile([P, 2, D], FP8, name=f"wq{j}")
                nc.sync.dma_start(out=t, in_=wq8[j])
                wq_sb.append(t)
                t = pp.tile([P, 2, D], FP8, name=f"wk{j}")
                nc.scalar.dma_start(out=t, in_=wk8[j])
                wk_sb.append(t)
                t = pp.tile([P, 2, D], FP8, name=f"wv{j}")
                nc.gpsimd.dma_start(out=t, in_=wv8[j])
                wv_sb.append(t)
            wo_sb = []
            for c in range(KC):
                t = pp.tile([P, D], BF16, name=f"wo{c}")
                nc.gpsimd.dma_start(out=t, in_=wo[c * P : (c + 1) * P, :])
                wo_sb.append(t)
            bq_sb, bk_sb = [], []
            for c in range(KC):
                t = pp.tile([P, 1], F32, name=f"bq{c}")
                nc.gpsimd.dma_start(out=t, in_=bq[c * P : (c + 1) * P].unsqueeze(1))
                bq_sb.append(t)
                t = pp.tile([P, 1], F32, name=f"bk{c}")
                nc.gpsimd.dma_start(out=t, in_=bk[c * P : (c + 1) * P].unsqueeze(1))
                bk_sb.append(t)
            bv_sb = pp.tile([1, D], BF16, name="bv")
            nc.gpsimd.dma_start(out=bv_sb, in_=bv)
            bo_sb = pp.tile([1, D], BF16, name="bo")
            nc.gpsimd.dma_start(out=bo_sb, in_=bo)
            gamma_sb = pp.tile([P, D], F32, name="gamma")
            nc.gpsimd.dma_start(out=gamma_sb, in_=gamma.unsqueeze(0).broadcast_to([P, D]))
            beta_sb = pp.tile([P, D], F32, name="beta")
            nc.gpsimd.dma_start(out=beta_sb, in_=beta.unsqueeze(0).broadcast_to([P, D]))
            eps_sb = pp.tile([P, 1], F32, name="eps")
            nc.vector.memset(eps_sb, EPS)
            ones_r = pp.tile([1, P], BF16, name="ones_r")
            nc.vector.memset(ones_r, 1.0)
            ident_sb = pp.tile([P, P], F32R, name="ident")
            nc.sync.dma_start(out=ident_sb, in_=ident)

            # ---- phase A: fp8 kc-paired k-major input loads ----
            QT8, KT8, VT8 = [], [], []
            for lst, srcap, nm, eng in (
                (QT8, qb8, "QT", nc.sync),
                (KT8, kb8, "KT", nc.scalar),
                (VT8, vb8, "VT", nc.gpsimd),
            ):
                for j in range(2):
                    t = pp.tile([P, 2, S], FP8, name=f"{nm}{j}")
                    eng.dma_start(out=t, in_=srcap[j])
                    lst.append(t)

            # residual prefetch (consumed in phase D)
            qres = []
            for st in range(ST):
                t = pp.tile([P, D], F32R, name=f"qres{st}")
                nc.sync.dma_start(out=t, in_=qf[st * P : (st + 1) * P, :])
                qres.append(t)

            # ---- phase B: projections (fp8 DoubleRow) ----
            qTp = [pp.tile([P, S], BF16, name=f"qTp{c}") for c in range(KC)]
            kTp = [pp.tile([P, S], BF16, name=f"kTp{c}") for c in range(KC)]
            # vbf[t]: [128t, 8 head blocks x 128] bf16
            #   even head block: [dv(64) | ones(64)]; odd: [ones | dv]
            # The ones columns fold the softmax denominator into the ctx
            # matmul (cost is per-output-column, so den rows are free).
            vbf = [pp.tile([P, H, P], BF16, name=f"vbf{t}") for t in range(TC)]
            for t in range(TC):
                nc.vector.memset(vbf[t], 1.0)

            with tc.tile_pool(name="psum_b", bufs=4, space="PSUM") as ppool:
                for c in range(KC):
                    csl = bass.ts(c, P)
                    for sbh in range(NSB):
                        ssl = bass.ts(sbh, SBW)
                        pq = ppool.tile([P, SBW], F32, name="proj")
                        for j in range(2):
                            nc.tensor.matmul(
                                pq,
                                lhsT=wq_sb[j][:, :, csl],
                                rhs=QT8[j][:, :, ssl],
                                start=(j == 0),
                                stop=(j == 1),
                                perf_mode=PM.DoubleRow,
                            )
                        nc.scalar.activation(
                            out=qTp[c][:, ssl], in_=pq, func=AF.Identity, bias=bq_sb[c]
                        )
                        pk = ppool.tile([P, SBW], F32, name="proj")
                        for j in range(2):
                            nc.tensor.matmul(
                                pk,
                                lhsT=wk_sb[j][:, :, csl],
                                rhs=KT8[j][:, :, ssl],
                                start=(j == 0),
                                stop=(j == 1),
                                perf_mode=PM.DoubleRow,
                            )
                        nc.vector.tensor_scalar(
                            out=kTp[c][:, ssl],
                            in0=pk,
                            scalar1=bk_sb[c],
                            scalar2=None,
                            op0=OP.add,
                        )
                for t in range(TC):
                    pv = ppool.tile([P, 4, 2, 64], F32, name="pv")
                    for j in range(2):
                        nc.tensor.matmul(
                            pv,
                            lhsT=VT8[j][:, :, bass.ts(t, P)],
                            rhs=wv_sb[j],
                            start=(j == 0),
                            stop=False,
                            perf_mode=PM.DoubleRow,
                        )
                    nc.tensor.matmul(pv, lhsT=ones_r, rhs=bv_sb, start=False, stop=True)
                    v8r = vbf[t].rearrange("p (a b) c -> p a b c", a=4, b=2)
                    nc.vector.tensor_copy(out=v8r[:, :, 0:1, 0:64], in_=pv[:, :, 0:1, :])
                    nc.vector.tensor_copy(out=v8r[:, :, 1:2, 64:128], in_=pv[:, :, 1:2, :])

            # ---- phase C: attention (per head pair p: heads 2p, 2p+1) ----
            ctxT = [pp.tile([P, S], BF16, name=f"ctxT{c}") for c in range(KC)]

            with (
                tc.tile_pool(name="psum_sc", bufs=2, space="PSUM") as psc,
                tc.tile_pool(name="psum_cd", bufs=2, space="PSUM") as pcd,
                tc.tile_pool(name="attn", bufs=3) as apool,
                tc.tile_pool(name="denr", bufs=2) as dpool,
            ):
                def _boundary(bp, bsb, bctx2):
                    # drain a finished (pair, s-block): den rows -> DRAM
                    # reshaped [128,8], reciprocal on 128 lanes, roundtrip
                    # back as partition-broadcast, normalize into ctxT.
                    # Called 2 t-pairs into the NEXT unit so this work queues
                    # BEHIND that unit's first exps and doesn't stall them.
                    u = bp * NSB + bsb
                    bsl = bass.ts(bsb, SBW)
                    # den rows to SBUF (DMA cannot read PSUM): A den at row
                    # 64 cols 0:512, B den at row 0 cols 512:1024.
                    stage = dpool.tile([P, 2 * SBW], F32, name="stage")
                    nc.scalar.activation(
                        out=stage[64:65, 0:SBW],
                        in_=bctx2[64:65, 0:SBW],
                        func=AF.Identity,
                    )
                    nc.scalar.activation(
                        out=stage[0:1, SBW : 2 * SBW],
                        in_=bctx2[0:1, SBW : 2 * SBW],
                        func=AF.Identity,
                    )
                    nc.gpsimd.dma_start(
                        out=dscr[u, 0:64, :].rearrange("p c -> (p c)").unsqueeze(0),
                        in_=stage[64:65, 0:SBW],
                    )
                    nc.gpsimd.dma_start(
                        out=dscr[u, 64:P, :].rearrange("p c -> (p c)").unsqueeze(0),
                        in_=stage[0:1, SBW : 2 * SBW],
                    )
                    recd = dpool.tile([P, 8], F32, name="recd")
                    nc.gpsimd.dma_start(out=recd, in_=dscr[u])
                    recr = dpool.tile([P, 8], F32, name="recr")
                    nc.vector.reciprocal(out=recr, in_=recd)
                    nc.gpsimd.dma_start(out=dscr2[u], in_=recr)
                    rbs = dpool.tile([P, SBW], F32, name="rbs")
                    nc.gpsimd.dma_start(
                        out=rbs[0:64, :],
                        in_=dscr2[u, 0:64, :]
                        .rearrange("p c -> (p c)")
                        .unsqueeze(0)
                        .broadcast_to([64, SBW]),
                    )
                    nc.gpsimd.dma_start(
                        out=rbs[64:P, :],
                        in_=dscr2[u, 64:P, :]
                        .rearrange("p c -> (p c)")
                        .unsqueeze(0)
                        .broadcast_to([64, SBW]),
                    )
                    nc.vector.tensor_tensor(
                        out=ctxT[bp][0:64, bsl],
                        in0=bctx2[0:64, 0:SBW],
                        in1=rbs[0:64, :],
                        op=OP.mult,
                    )
                    nc.vector.tensor_tensor(
                        out=ctxT[bp][64:P, bsl],
                        in0=bctx2[64:P, SBW : 2 * SBW],
                        in1=rbs[64:P, :],
                        op=OP.mult,
                    )

                _pending = None
                for p in range(KC):
                    for sb in range(NSB):
                        ssl = bass.ts(sb, SBW)
                        ctx2 = pcd.tile([P, 2 * SBW], F32, name="ctx2")
                        for i in range(TP):
                            if i == 2 and _pending is not None:
                                _boundary(*_pending)
                                _pending = None
                            eA = apool.tile([P, 2, SBW], BF16, name="eA")
                            eB = apool.tile([P, 2, SBW], BF16, name="eB")
                            for j in range(2):
                                t = 2 * i + j
                                tsl = bass.ts(t, P)
                                sc2 = psc.tile([P, 2 * SBW], F32, name="sc2")
                                nc.tensor.matmul(
                                    sc2[:, 0:SBW],
                                    lhsT=kTp[p][0:DK, tsl],
                                    rhs=qTp[p][0:DK, ssl],
                                    start=True,
                                    stop=True,
                                    tile_position=(0, 0),
                                )
                                nc.tensor.matmul(
                                    sc2[:, SBW : 2 * SBW],
                                    lhsT=kTp[p][DK:P, tsl],
                                    rhs=qTp[p][DK:P, ssl],
                                    start=True,
                                    stop=True,
                                    tile_position=(64, 0),
                                )
                                nc.scalar.activation(
                                    out=eA[:, j, :],
                                    in_=sc2[:, 0:SBW],
                                    func=AF.Exp,
                                    scale=SCALE,
                                )
                                nc.vector.tensor_scalar(
                                    out=eB[:, j, :].bitcast(I16),
                                    in0=sc2[:, SBW : 2 * SBW],
                                    scalar1=SCH_S,
                                    scalar2=SCH_B,
                                    op0=OP.mult,
                                    op1=OP.add,
                                )
                                nc.tensor.matmul(
                                    ctx2[:, 0:SBW],
                                    lhsT=vbf[t][:, 2 * p, :],
                                    rhs=eA[:, j, :],
                                    start=(t == 0),
                                    stop=(t == TC - 1),
                                )
                                nc.tensor.matmul(
                                    ctx2[:, SBW : 2 * SBW],
                                    lhsT=vbf[t][:, 2 * p + 1, :],
                                    rhs=eB[:, j, :],
                                    start=(t == 0),
                                    stop=(t == TC - 1),
                                )
                        _pending = (p, sb, ctx2)

                if _pending is not None:
                    _boundary(*_pending)
                    _pending = None

            # ---- phase D: output projection, residual, LN ----
            with (
                tc.tile_pool(name="psum_o", bufs=3, space="PSUM") as pout,
                tc.tile_pool(name="work", bufs=3) as wpool,
            ):
                for st in range(ST):
                    stsl = bass.ts(st, P)
                    po = pout.tile([P, D], F32, name="pout")
                    for c in range(KC):
                        nc.tensor.matmul(
                            po,
                            lhsT=ctxT[c][:, stsl],
                            rhs=wo_sb[c],
                            start=(c == 0),
                            stop=False,
                        )
                    nc.tensor.matmul(
                        po, lhsT=ones_r, rhs=bo_sb, start=False, stop=False
                    )
                    nc.tensor.matmul(
                        po,
                        lhsT=ident_sb[:],
                        rhs=qres[st][:],
                        start=False,
                        stop=True,
                    )
                    stats = wpool.tile([P, 6], F32, name="stats")
                    nc.vector.bn_stats(out=stats, in_=po)
                    mv = wpool.tile([P, 2], F32, name="mv")
                    nc.vector.bn_aggr(out=mv, in_=stats)
                    sq = wpool.tile([P, 1], F32, name="sq")
                    nc.scalar.activation(
                        out=sq, in_=mv[:, 1:2], func=AF.Sqrt, bias=eps_sb
                    )
                    rstd = wpool.tile([P, 1], F32, name="rstd")
                    nc.vector.reciprocal(out=rstd, in_=sq)
                    negmu = wpool.tile([P, 1], F32, name="negmu")
                    nc.vector.tensor_scalar(
                        out=negmu,
                        in0=mv[:, 0:1],
                        scalar1=rstd,
                        scalar2=-1.0,
                        op0=OP.mult,
                        op1=OP.mult,
                    )
                    x = wpool.tile([P, D], F32, name="x")
                    nc.scalar.activation(
                        out=x, in_=po, func=AF.Identity, bias=negmu, scale=rstd
                    )
                    nc.vector.tensor_mul(out=x, in0=x, in1=gamma_sb)
                    nc.gpsimd.tensor_tensor(
                        out=x, in0=x, in1=beta_sb, op=OP.add
                    )
                    nc.sync.dma_start(out=out[st * P : (st + 1) * P, :], in_=x)

    _split_excess_waits(nc)
    return nc


_NC_CACHE = None


def _get_program():
    global _NC_CACHE
    if _NC_CACHE is None:
        _NC_CACHE = build_program()
    return _NC_CACHE


def _pair_kc(xT):
    """[D, S]-like array -> kc-paired [2, 128, 2, S] layout."""
    d, s = xT.shape
    return np.ascontiguousarray(
        xT.reshape(2, 2, P, s).transpose(0, 2, 1, 3)
    )


def make_in_maps(Q, K, V, Wq, bq, Wk, bk, Wv, bv, Wo, bo, gamma, beta):
    bf = ml_dtypes.bfloat16
    f8 = ml_dtypes.float8_e4m3fn
    Q = np.asarray(Q, np.float32)
    K = np.asarray(K, np.float32)
    V = np.asarray(V, np.float32)
    wq8 = _pair_kc(np.asarray(Wq, np.float32).T).astype(f8)
    wk8 = _pair_kc(np.asarray(Wk, np.float32).T).astype(f8)
    wv8 = _pair_kc(np.asarray(Wv, np.float32).T).astype(f8)
    woT = np.ascontiguousarray(np.asarray(Wo, np.float32).T).astype(bf)
    bv_r = np.asarray(bv, np.float32).reshape(1, D).astype(bf)
    bo_r = np.asarray(bo, np.float32).reshape(1, D).astype(bf)
    ident = np.eye(P, dtype=np.float32)
    in_maps = []
    for b in range(N_CORES):
        in_maps.append(
            {
                "qf": np.ascontiguousarray(Q[b]),
                "qb8": _pair_kc(Q[b].T).astype(f8),
                "kb8": _pair_kc(K[b].T).astype(f8),
                "vb8": _pair_kc(V[b].T).astype(f8),
                "wq8": wq8,
                "wk8": wk8,
                "wv8": wv8,
                "wo": woT,
                "bq": np.asarray(bq, np.float32),
                "bk": np.asarray(bk, np.float32),
                "bv": bv_r,
                "bo": bo_r,
                "ident": ident,
                "gamma": np.asarray(gamma, np.float32),
                "beta": np.asarray(beta, np.float32),
            }
        )
    return in_maps


def run(in_maps, trace=False, **kw):
    nc = _get_program()
    return run_bass_kernel_spmd(
        nc, in_maps, core_ids=list(range(N_CORES)), trace=trace, **kw
    )


def kernel(**inputs):
    in_maps = make_in_maps(**inputs)
    res = run(in_maps)
    out = np.stack([res.results[b]["out"] for b in range(N_CORES)], axis=0)
    return out.astype(np.float32)


# revision 21
# speedup vs baseline: 1.3719x; 1.0288x over previous
"""Multi-head attention + residual + LayerNorm, Trainium2 Bass kernel.

Problem (hardcoded): B=8, S=2048, D=512, H=8, DK=64, fp32 I/O.
  q = Q@Wq.T+bq; k = K@Wk.T+bk; v = V@Wv.T+bv        (per batch, split 8 heads)
  attn = softmax(q k^T / sqrt(DK)); ctx = attn @ v
  out = LayerNorm(ctx@Wo.T + bo + Q) * gamma + beta

Sharding: pure data-parallel over batch: core b handles batch element b
(B == n_cores == 8), no collectives.

Per-core dataflow (t-major attention, fp8 DoubleRow matmuls, fp32 LN):
  - Q/K/V pre-transposed AND fp8(e4m3)-cast on host to kc-paired k-major
    layout [2, 128, 2, S]; projections run as fp8 DoubleRow matmuls
    (K=256 contraction per instruction = 2x bf16 column throughput).
  - qT,kT projections output bf16 [d_out, s]; v outputs to v8p fp8 tiles
    [128t, 2(t-parity), 8*128] where each head block is [dv(64)|ones(64)]
    for even heads and [ones|dv] for odd heads.  The ones columns fold
    the softmax denominator into the ctx matmul (cost is per-output-
    column, so 64 redundant den rows are free).
  - Attention per head pair p, s-block (512):
      scoresT[t,s] bf16 matmuls into a shared [128,1024] PSUM pair
      (head A cols 0:512, head B 512:1024),
      exp: head A true exp on ACT -> fp8, head B Schraudolph int8
      bit-trick on DVE -> fp8,
      ctx+den: ONE fp8 DoubleRow matmul per head per t-chunk-PAIR
      (contraction 256 t's) accumulating [dv|den] in PSUM.
  - Boundary per (pair, s-block): reciprocal of den row on DVE,
    PE outer-product broadcasts 1/den into the other head's dead den
    rows (partition-aligned), Pool-engine multiply normalizes into
    bf16 ctxT.  No DRAM roundtrips.
  - Output projection, +bias, +residual (fp32r identity matmul on
    prefetched Q tiles), LayerNorm, gamma/beta, DMA out.

Toolchain workarounds: this walrus build caps sem-waits per instruction
at 1 (excess waits hoisted onto same-engine NOPs).
"""

import numpy as np
import ml_dtypes

import bass_rust
import concourse.bass as bass
import concourse.mybir as mybir
import concourse.tile as tile
from concourse.bass_utils import run_bass_kernel_spmd
from concourse.vector_clock import ScopedClock

F32 = mybir.dt.float32
F32R = mybir.dt.float32r
BF16 = mybir.dt.bfloat16
FP8 = mybir.dt.float8e4
I8 = mybir.dt.int8
I16 = mybir.dt.int16
AF = mybir.ActivationFunctionType
OP = mybir.AluOpType
PM = mybir.MatmulPerfMode

N_CORES = 8
S, D, H, DK = 2048, 512, 8, 64
P = 128
KC = D // P        # 4 contraction chunks
TC = S // P        # 16 t-chunks
TP = TC // 2       # 8 t-chunk pairs
ST = S // P        # 16 s-tiles (output)
SBW = 512          # attention s-block width
NSB = S // SBW     # 4
EPS = 1e-5
SCALE = 1.0 / np.sqrt(DK)

# Schraudolph exp in bf16-bit space: bits = round(x*L*SCALE + (16256 - C))
SCH_L = 128.0 / np.log(2.0)
SCH_C = 5.60
SCH_S = float(SCALE * SCH_L)
SCH_B = float(16256.0 - SCH_C)

_MAX_CTRL_WAITS = 1


def _patch_tile_tail():
    """walrus in this toolchain rejects >1 sem wait on CTRL instructions
    (Drain/NoOp). Move the Tile tail-drain's waits onto a chain of NOPs,
    one wait each."""
    if getattr(tile.TileContext, "_tail_patched", False):
        return

    def _patched(self, tick_clock, wait_clock):
        nc = self.nc
        scratch = nc.sync.nop(nofuse=True, hint="tail_wait")
        wait_clock.add_sem_waits(
            scratch.ins, ScopedClock({None: tick_clock.global_clock})
        )
        si = scratch.ins.sync_info
        waits = list(si.on_wait) if si is not None else []
        if len(waits) > _MAX_CTRL_WAITS:
            scratch.ins.sync_info = bass_rust.SyncInfo(
                on_wait=waits[:_MAX_CTRL_WAITS], on_update=list(si.on_update)
            )
            for i in range(_MAX_CTRL_WAITS, len(waits), _MAX_CTRL_WAITS):
                extra = nc.sync.nop(nofuse=True, hint=f"tail_wait_{i}")
                extra.ins.sync_info = bass_rust.SyncInfo(
                    on_wait=waits[i : i + _MAX_CTRL_WAITS], on_update=[]
                )
        nc.sync.drain()
        nc.all_engine_barrier()
        popped = nc._tile_sem_poison_stack.pop()
        assert popped is self._sem_poison
        nc.clear_and_free_semaphores(list(self.sems.allocated().values()))
        nc.all_engine_barrier()

    tile.TileContext._drain_and_barrier = _patched
    tile.TileContext._tail_patched = True


def _split_excess_waits(nc, max_waits=_MAX_CTRL_WAITS):
    """walrus (this build) caps sem waits per instruction very low. Hoist
    excess waits onto same-engine NOPs inserted just before the instruction
    (same queue, in order — semantically identical)."""
    def make_nop(engine, waits):
        bi = nc.engines[engine].nop(nofuse=True, hint="waitsplit")
        nop_inst = bi.ins
        cur = nc.cur_bb.bb
        lst = list(cur.instructions)
        assert lst and lst[-1].name == nop_inst.name
        lst.pop()
        cur.instructions = lst
        nop_inst.sync_info = bass_rust.SyncInfo(on_wait=waits, on_update=[])
        return nop_inst

    ctr = 0
    for f in nc.m.functions:
        for bb in f.blocks:
            old = list(bb.instructions)
            new = []
            changed = False
            for inst in old:
                si = inst.sync_info
                waits = list(si.on_wait) if si is not None else []
                if len(waits) > max_waits:
                    changed = True
                    excess, keep = waits[:-max_waits], waits[-max_waits:]
                    for i in range(0, len(excess), max_waits):
                        ctr += 1
                        new.append(make_nop(inst.engine, excess[i : i + max_waits]))
                    inst.sync_info = bass_rust.SyncInfo(
                        on_wait=keep, on_update=list(si.on_update)
                    )
                new.append(inst)
            if changed:
                bb.instructions = new
    return ctr


_LDW_OPT = False


def _patch_ldw_opt():
    """Enable walrus's LDWEIGHTS pull-ahead (background weight buffer) —
    concourse pins it off, but it is a large win for a LDW-per-matmul
    instruction stream. Correctness is re-verified against the reference
    each run."""
    import concourse.bass_utils as bu

    if getattr(bu, "_ldw_patched", False):
        return
    orig = bu.run_command

    def patched(cmd, **kw):
        if _LDW_OPT and isinstance(cmd, list):
            cmd = [
                c.replace("--enable-ldw-opt=false", "--enable-ldw-opt=true")
                if isinstance(c, str)
                else c
                for c in cmd
            ]
        return orig(cmd, **kw)

    bu.run_command = patched
    bu._ldw_patched = True


def build_program():
    _patch_tile_tail()
    _patch_ldw_opt()
    nc = bass.Bass("TRN2", target_bir_lowering=False, debug=False, num_devices=1)

    qf = nc.dram_tensor("qf", (S, D), F32R, kind="ExternalInput").ap()
    qb8 = nc.dram_tensor("qb8", (2, P, 2, S), FP8, kind="ExternalInput").ap()
    kb8 = nc.dram_tensor("kb8", (2, P, 2, S), FP8, kind="ExternalInput").ap()
    vb8 = nc.dram_tensor("vb8", (2, P, 2, S), FP8, kind="ExternalInput").ap()
    wq8 = nc.dram_tensor("wq8", (2, P, 2, D), FP8, kind="ExternalInput").ap()
    wk8 = nc.dram_tensor("wk8", (2, P, 2, D), FP8, kind="ExternalInput").ap()
    wv8 = nc.dram_tensor("wv8", (2, P, 2, D), FP8, kind="ExternalInput").ap()
    wo = nc.dram_tensor("wo", (D, D), BF16, kind="ExternalInput").ap()
    bq = nc.dram_tensor("bq", (D,), F32, kind="ExternalInput").ap()
    bk = nc.dram_tensor("bk", (D,), F32, kind="ExternalInput").ap()
    bv = nc.dram_tensor("bv", (1, D), BF16, kind="ExternalInput").ap()
    bo = nc.dram_tensor("bo", (1, D), BF16, kind="ExternalInput").ap()
    gamma = nc.dram_tensor("gamma", (D,), F32, kind="ExternalInput").ap()
    beta = nc.dram_tensor("beta", (D,), F32, kind="ExternalInput").ap()
    ident = nc.dram_tensor("ident", (P, P), F32R, kind="ExternalInput").ap()
    out = nc.dram_tensor("out", (S, D), F32, kind="ExternalOutput").ap()
    # DRAM scratch for per-(pair, s-block) softmax denominators: written as
    # two [1,512] rows, read back as [128,8] (so the reciprocal runs on 128
    # DVE lanes instead of 1), recip written out, then partition-broadcast
    # back in (broadcast DMA needs a DRAM source).
    dscr = nc.dram_tensor("dscr", (KC * NSB, P, 8), F32, kind="Internal").ap()
    dscr2 = nc.dram_tensor("dscr2", (KC * NSB, P, 8), F32, kind="Internal").ap()

    with tile.TileContext(nc) as tc:
        with tc.tile_pool(name="persist", bufs=1) as pp:
            # ---- constants / weights ----
            wq_sb, wk_sb, wv_sb = [], [], []
            for j in range(2):
                t = pp.tile([P, 2, D], FP8, name=f"wq{j}")
                nc.sync.dma_start(out=t, in_=wq8[j])
                wq_sb.append(t)
                t = pp.tile([P, 2, D], FP8, name=f"wk{j}")
                nc.scalar.dma_start(out=t, in_=wk8[j])
                wk_sb.append(t)
                t = pp.tile([P, 2, D], FP8, name=f"wv{j}")
                nc.gpsimd.dma_start(out=t, in_=wv8[j])
                wv_sb.append(t)
            wo_sb = []
            for c in range(KC):
                t = pp.tile([P, D], BF16, name=f"wo{c}")
                nc.gpsimd.dma_start(out=t, in_=wo[c * P : (c + 1) * P, :])
                wo_sb.append(t)
            bq_sb, bk_sb = [], []
            for c in range(KC):
                t = pp.tile([P, 1], F32, name=f"bq{c}")
                nc.gpsimd.dma_start(out=t, in_=bq[c * P : (c + 1) * P].unsqueeze(1))
                bq_sb.append(t)
                t = pp.tile([P, 1], F32, name=f"bk{c}")
                nc.gpsimd.dma_start(out=t, in_=bk[c * P : (c + 1) * P].unsqueeze(1))
                bk_sb.append(t)
            bv_sb = pp.tile([1, D], BF16, name="bv")
            nc.gpsimd.dma_start(out=bv_sb, in_=bv)
            bo_sb = pp.tile([1, D], BF16, name="bo")
            nc.gpsimd.dma_start(out=bo_sb, in_=bo)
            gamma_sb = pp.tile([P, D], F32, name="gamma")
            nc.gpsimd.dma_start(out=gamma_sb, in_=gamma.unsqueeze(0).broadcast_to([P, D]))
            beta_sb = pp.tile([P, D], F32, name="beta")
            nc.gpsimd.dma_start(out=beta_sb, in_=beta.unsqueeze(0).broadcast_to([P, D]))
            eps_sb = pp.tile([P, 1], F32, name="eps")
            nc.vector.memset(eps_sb, EPS)
            ones_r = pp.tile([1, P], BF16, name="ones_r")
            nc.vector.memset(ones_r, 1.0)
            ident_sb = pp.tile([P, P], F32R, name="ident")
            nc.sync.dma_start(out=ident_sb, in_=ident)

            # ---- phase A: fp8 kc-paired k-major input loads ----
            QT8, KT8, VT8 = [], [], []
            for lst, srcap, nm, eng in (
                (QT8, qb8, "QT", nc.sync),
                (KT8, kb8, "KT", nc.scalar),
                (VT8, vb8, "VT", nc.gpsimd),
            ):
                for j in range(2):
                    t = pp.tile([P, 2, S], FP8, name=f"{nm}{j}")
                    eng.dma_start(out=t, in_=srcap[j])
                    lst.append(t)

            # residual prefetch (consumed in phase D)
            qres = []
            for st in range(ST):
                t = pp.tile([P, D], F32R, name=f"qres{st}")
                nc.sync.dma_start(out=t, in_=qf[st * P : (st + 1) * P, :])
                qres.append(t)

            # ---- phase B: projections (fp8 DoubleRow) ----
            qTp = [pp.tile([P, S], BF16, name=f"qTp{c}") for c in range(KC)]
            kTp = [pp.tile([P, S], BF16, name=f"kTp{c}") for c in range(KC)]
            # vbf[t]: [128t, 8 head blocks x 128] bf16
            #   even head block: [dv(64) | ones(64)]; odd: [ones | dv]
            # The ones columns fold the softmax denominator into the ctx
            # matmul (cost is per-output-column, so den rows are free).
            vbf = [pp.tile([P, H, P], BF16, name=f"vbf{t}") for t in range(TC)]
            for t in range(TC):
                nc.vector.memset(vbf[t], 1.0)

            with tc.tile_pool(name="psum_b", bufs=4, space="PSUM") as ppool:
                for c in range(KC):
                    csl = bass.ts(c, P)
                    for sbh in range(NSB):
                        ssl = bass.ts(sbh, SBW)
                        pq = ppool.tile([P, SBW], F32, name="proj")
                        for j in range(2):
                            nc.tensor.matmul(
                                pq,
                                lhsT=wq_sb[j][:, :, csl],
                                rhs=QT8[j][:, :, ssl],
                                start=(j == 0),
                                stop=(j == 1),
                                perf_mode=PM.DoubleRow,
                            )
                        nc.scalar.activation(
                            out=qTp[c][:, ssl], in_=pq, func=AF.Identity, bias=bq_sb[c]
                        )
                        pk = ppool.tile([P, SBW], F32, name="proj")
                        for j in range(2):
                            nc.tensor.matmul(
                                pk,
                                lhsT=wk_sb[j][:, :, csl],
                                rhs=KT8[j][:, :, ssl],
                                start=(j == 0),
                                stop=(j == 1),
                                perf_mode=PM.DoubleRow,
                            )
                        nc.vector.tensor_scalar(
                            out=kTp[c][:, ssl],
                            in0=pk,
                            scalar1=bk_sb[c],
                            scalar2=None,
                            op0=OP.add,
                        )
                for t in range(TC):
                    pv = ppool.tile([P, 4, 2, 64], F32, name="pv")
                    for j in range(2):
                        nc.tensor.matmul(
                            pv,
                            lhsT=VT8[j][:, :, bass.ts(t, P)],
                            rhs=wv_sb[j],
                            start=(j == 0),
                            stop=False,
                            perf_mode=PM.DoubleRow,
                        )
                    nc.tensor.matmul(pv, lhsT=ones_r, rhs=bv_sb, start=False, stop=True)
                    v8r = vbf[t].rearrange("p (a b) c -> p a b c", a=4, b=2)
                    nc.vector.tensor_copy(out=v8r[:, :, 0:1, 0:64], in_=pv[:, :, 0:1, :])
                    nc.vector.tensor_copy(out=v8r[:, :, 1:2, 64:128], in_=pv[:, :, 1:2, :])

            # ---- phase C: attention (per head pair p: heads 2p, 2p+1) ----
            ctxT = [pp.tile([P, S], BF16, name=f"ctxT{c}") for c in range(KC)]

            with (
                tc.tile_pool(name="psum_sc", bufs=2, space="PSUM") as psc,
                tc.tile_pool(name="psum_cd", bufs=2, space="PSUM") as pcd,
                tc.tile_pool(name="attn", bufs=3) as apool,
                tc.tile_pool(name="denr", bufs=2) as dpool,
            ):
                def _boundary(bp, bsb, bctx2):
                    # drain a finished (pair, s-block): den rows -> DRAM
                    # reshaped [128,8], reciprocal on 128 lanes, roundtrip
                    # back as partition-broadcast, normalize into ctxT.
                    # Called 2 t-pairs into the NEXT unit so this work queues
                    # BEHIND that unit's first exps and doesn't stall them.
                    u = bp * NSB + bsb
                    bsl = bass.ts(bsb, SBW)
                    # den rows to SBUF (DMA cannot read PSUM): A den at row
                    # 64 cols 0:512, B den at row 0 cols 512:1024.
                    stage = dpool.tile([P, 2 * SBW], F32, name="stage")
                    nc.scalar.activation(
                        out=stage[64:65, 0:SBW],
                        in_=bctx2[64:65, 0:SBW],
                        func=AF.Identity,
                    )
                    nc.scalar.activation(
                        out=stage[0:1, SBW : 2 * SBW],
                        in_=bctx2[0:1, SBW : 2 * SBW],
                        func=AF.Identity,
                    )
                    nc.gpsimd.dma_start(
                        out=dscr[u, 0:64, :].rearrange("p c -> (p c)").unsqueeze(0),
                        in_=stage[64:65, 0:SBW],
                    )
                    nc.gpsimd.dma_start(
                        out=dscr[u, 64:P, :].rearrange("p c -> (p c)").unsqueeze(0),
                        in_=stage[0:1, SBW : 2 * SBW],
                    )
                    recd = dpool.tile([P, 8], F32, name="recd")
                    nc.gpsimd.dma_start(out=recd, in_=dscr[u])
                    recr = dpool.tile([P, 8], F32, name="recr")
                    nc.vector.reciprocal(out=recr, in_=recd)
                    nc.gpsimd.dma_start(out=dscr2[u], in_=recr)
                    rbs = dpool.tile([P, SBW], F32, name="rbs")
                    nc.gpsimd.dma_start(
                        out=rbs[0:64, :],
                        in_=dscr2[u, 0:64, :]
                        .rearrange("p c -> (p c)")
                        .unsqueeze(0)
                        .broadcast_to([64, SBW]),
                    )
                    nc.gpsimd.dma_start(
                        out=rbs[64:P, :],
                        in_=dscr2[u, 64:P, :]
                        .rearrange("p c -> (p c)")
                        .unsqueeze(0)
                        .broadcast_to([64, SBW]),
                    )
                    nc.vector.tensor_tensor(
                        out=ctxT[bp][0:64, bsl],
                        in0=bctx2[0:64, 0:SBW],
                        in1=rbs[0:64, :],
                        op=OP.mult,
                    )
                    nc.vector.tensor_tensor(
                        out=ctxT[bp][64:P, bsl],
                        in0=bctx2[64:P, SBW : 2 * SBW],
                        in1=rbs[64:P, :],
                        op=OP.mult,
                    )

                _pending = None
                for p in range(KC):
                    for sb in range(NSB):
                        ssl = bass.ts(sb, SBW)
                        ctx2 = pcd.tile([P, 2 * SBW], F32, name="ctx2")
                        # ctx matmuls trail the scores by one t-chunk so the
                        # PE never waits on the exp of the chunk it just
                        # scored (one-deep software pipeline).
                        _pctx = None

                        def _flush(p=p, ctx2=ctx2):
                            t, fA, fB = _pctx
                            nc.tensor.matmul(
                                ctx2[:, 0:SBW],
                                lhsT=vbf[t][:, 2 * p, :],
                                rhs=fA,
                                start=(t == 0),
                                stop=(t == TC - 1),
                            )
                            nc.tensor.matmul(
                                ctx2[:, SBW : 2 * SBW],
                                lhsT=vbf[t][:, 2 * p + 1, :],
                                rhs=fB,
                                start=(t == 0),
                                stop=(t == TC - 1),
                            )

                        for t in range(TC):
                            if t == 4 and _pending is not None:
                                _boundary(*_pending)
                                _pending = None
                            tsl = bass.ts(t, P)
                            sc2 = psc.tile([P, 2 * SBW], F32, name="sc2")
                            nc.tensor.matmul(
                                sc2[:, 0:SBW],
                                lhsT=kTp[p][0:DK, tsl],
                                rhs=qTp[p][0:DK, ssl],
                                start=True,
                                stop=True,
                                tile_position=(0, 0),
                            )
                            nc.tensor.matmul(
                                sc2[:, SBW : 2 * SBW],
                                lhsT=kTp[p][DK:P, tsl],
                                rhs=qTp[p][DK:P, ssl],
                                start=True,
                                stop=True,
                                tile_position=(64, 0),
                            )
                            if _pctx is not None:
                                _flush()
                            eA = apool.tile([P, SBW], BF16, name="eA")
                            eB = apool.tile([P, SBW], BF16, name="eB")
                            nc.scalar.activation(
                                out=eA,
                                in_=sc2[:, 0:SBW],
                                func=AF.Exp,
                                scale=SCALE,
                            )
                            nc.vector.tensor_scalar(
                                out=eB.bitcast(I16),
                                in0=sc2[:, SBW : 2 * SBW],
                                scalar1=SCH_S,
                                scalar2=SCH_B,
                                op0=OP.mult,
                                op1=OP.add,
                            )
                            _pctx = (t, eA, eB)
                        _flush()
                        _pctx = None
                        _pending = (p, sb, ctx2)

                if _pending is not None:
                    _boundary(*_pending)
                    _pending = None

            # ---- phase D: output projection, residual, LN ----
            with (
                tc.tile_pool(name="psum_o", bufs=3, space="PSUM") as pout,
                tc.tile_pool(name="work", bufs=3) as wpool,
            ):
                for st in range(ST):
                    stsl = bass.ts(st, P)
                    po = pout.tile([P, D], F32, name="pout")
                    for c in range(KC):
                        nc.tensor.matmul(
                            po,
                            lhsT=ctxT[c][:, stsl],
                            rhs=wo_sb[c],
                            start=(c == 0),
                            stop=False,
                        )
                    nc.tensor.matmul(
                        po, lhsT=ones_r, rhs=bo_sb, start=False, stop=False
                    )
                    nc.tensor.matmul(
                        po,
                        lhsT=ident_sb[:],
                        rhs=qres[st][:],
                        start=False,
                        stop=True,
                    )
                    stats = wpool.tile([P, 6], F32, name="stats")
                    nc.vector.bn_stats(out=stats, in_=po)
                    mv = wpool.tile([P, 2], F32, name="mv")
                    nc.vector.bn_aggr(out=mv, in_=stats)
                    sq = wpool.tile([P, 1], F32, name="sq")
                    nc.scalar.activation(
                        out=sq, in_=mv[:, 1:2], func=AF.Sqrt, bias=eps_sb
                    )
                    rstd = wpool.tile([P, 1], F32, name="rstd")
                    nc.vector.reciprocal(out=rstd, in_=sq)
                    negmu = wpool.tile([P, 1], F32, name="negmu")
                    nc.vector.tensor_scalar(
                        out=negmu,
                        in0=mv[:, 0:1],
                        scalar1=rstd,
                        scalar2=-1.0,
                        op0=OP.mult,
                        op1=OP.mult,
                    )
                    x = wpool.tile([P, D], F32, name="x")
                    nc.scalar.activation(
                        out=x, in_=po, func=AF.Identity, bias=negmu, scale=rstd
                    )
                    nc.vector.tensor_mul(out=x, in0=x, in1=gamma_sb)
                    nc.gpsimd.tensor_tensor(
                        out=x, in0=x, in1=beta_sb, op=OP.add
                    )
                    nc.sync.dma_start(out=out[st * P : (st + 1) * P, :], in_=x)

    _split_excess_waits(nc)
    return nc


_NC_CACHE = None


def _get_program():
    global _NC_CACHE
    if _NC_CACHE is None:
        _NC_CACHE = build_program()
    return _NC_CACHE


def _pair_kc(xT):
    """[D, S]-like array -> kc-paired [2, 128, 2, S] layout."""
    d, s = xT.shape
    return np.ascontiguousarray(
        xT.reshape(2, 2, P, s).transpose(0, 2, 1, 3)
    )


def make_in_maps(Q, K, V, Wq, bq, Wk, bk, Wv, bv, Wo, bo, gamma, beta):
    bf = ml_dtypes.bfloat16
    f8 = ml_dtypes.float8_e4m3fn
    Q = np.asarray(Q, np.float32)
    K = np.asarray(K, np.float32)
    V = np.asarray(V, np.float32)
    wq8 = _pair_kc(np.asarray(Wq, np.float32).T).astype(f8)
    wk8 = _pair_kc(np.asarray(Wk, np.float32).T).astype(f8)
    wv8 = _pair_kc(np.asarray(Wv, np.float32).T).astype(f8)
    woT = np.ascontiguousarray(np.asarray(Wo, np.float32).T).astype(bf)
    bv_r = np.asarray(bv, np.float32).reshape(1, D).astype(bf)
    bo_r = np.asarray(bo, np.float32).reshape(1, D).astype(bf)
    ident = np.eye(P, dtype=np.float32)
    in_maps = []
    for b in range(N_CORES):
        in_maps.append(
            {
                "qf": np.ascontiguousarray(Q[b]),
                "qb8": _pair_kc(Q[b].T).astype(f8),
                "kb8": _pair_kc(K[b].T).astype(f8),
                "vb8": _pair_kc(V[b].T).astype(f8),
                "wq8": wq8,
                "wk8": wk8,
                "wv8": wv8,
                "wo": woT,
                "bq": np.asarray(bq, np.float32),
                "bk": np.asarray(bk, np.float32),
                "bv": bv_r,
                "bo": bo_r,
                "ident": ident,
                "gamma": np.asarray(gamma, np.float32),
                "beta": np.asarray(beta, np.float32),
            }
        )
    return in_maps


def run(in_maps, trace=False, **kw):
    nc = _get_program()
    return run_bass_kernel_spmd(
        nc, in_maps, core_ids=list(range(N_CORES)), trace=trace, **kw
    )


def kernel(**inputs):
    in_maps = make_in_maps(**inputs)
    res = run(in_maps)
    out = np.stack([res.results[b]["out"] for b in range(N_CORES)], axis=0)
    return out.astype(np.float32)
